# revision 1
# baseline (speedup 1.0000x reference)
"""DiffPool-like GNN (two GCN convs + softmax clustering + weighted pooling)
as a Bass/Tile SPMD kernel on 8 Trainium2 NeuronCores.

Strategy (matches the sharding hint):
  * nodes partitioned into 8 contiguous shards; each core owns the edges whose
    dst falls in its shard (host buckets edges by 128-node dst window);
  * W1/W2 replicated; per-core f32 feature tables g = (D^-1/2 x) @ W are
    built on-device and stored in HBM in a per-core permuted row order (own
    shard first) so one SPMD program serves all cores;
  * per-edge messages fetched with the MoE dma_gather primitive (256B rows,
    int16 indices -> the table is split into 4 parts of Npad/4 rows, each with
    a trailing zero row for padding slots);
  * segment-sum on the tensor engine: per 128-slot chunk, a one-hot matrix
    M[p, r] = (dstloc[p] == r) is built on the vector engine and
    agg += M.T @ msgs accumulates in PSUM across the window's chunks;
  * conv output x = dinv*(agg + g_self) + b;
  * two launches: (A) conv1 -> x rows (and xs = dinv*x for the next table);
    host regroups; (B) conv2 -> softmax S -> weighted pooling via the same
    masked-matmul trick, emitting per-core partial pooled sums which the host
    adds (the "all-reduce").

The walrus build in this container encodes at most ONE sync wait per
instruction; _split_waits() rewrites the scheduled BIR, moving excess waits
onto injected single-wait NoOps.
"""

import os
import sys
import numpy as np

sys.path.insert(0, "/opt/trn_rl_repo")

import ml_dtypes  # noqa: E402
import concourse.bacc as bacc  # noqa: E402
import concourse.mybir as mybir  # noqa: E402
import concourse.tile as tile  # noqa: E402
from concourse.bass_utils import run_bass_kernel_spmd  # noqa: E402
from concourse.tile_rust import add_dep_helper  # noqa: E402

P = 128
BF16 = mybir.dt.bfloat16
F32 = mybir.dt.float32
I16 = mybir.dt.int16
I32 = mybir.dt.int32
NP_BF16 = ml_dtypes.bfloat16

AluOp = mybir.AluOpType
ActFn = mybir.ActivationFunctionType

_DT_MAP = {
    np.dtype(np.float32): F32,
    np.dtype(np.int16): I16,
    np.dtype(NP_BF16): BF16,
}

PARTS = 4


class ConstBundle:
    """Packs [128, n] arrays of mixed dtypes into one [128, W] int32 array."""

    def __init__(self):
        self.fields = {}
        self.nbytes = 0

    def add(self, name, dtype, n):
        dt = np.dtype(dtype)
        b = dt.itemsize * n
        b4 = (b + 3) & ~3
        self.fields[name] = (self.nbytes, dt, n)
        self.nbytes += b4

    def pack(self, arrays):
        w = self.nbytes // 4
        out = np.zeros((P, w), np.int32)
        ob = out.view(np.uint8)
        for name, (off, dt, n) in self.fields.items():
            a = np.ascontiguousarray(arrays[name])
            assert a.dtype == dt and a.shape == (P, n), (name, a.dtype, a.shape)
            ob[:, off:off + dt.itemsize * n] = a.view(np.uint8)
        return out

    def view(self, cb_sb, name):
        off, dt, n = self.fields[name]
        b4 = (dt.itemsize * n + 3) & ~3
        v = cb_sb[:, off // 4:(off + b4) // 4].bitcast(_DT_MAP[dt])
        return v[:, :n]


def _split_waits(nc, budget=1):
    """Move excess sync waits onto injected single-wait same-engine NoOps.
    The walrus in this container encodes at most one wait per instruction."""
    for fn in nc.m.functions:
        for blk in fn.blocks:
            out = []
            for ins in blk.instructions:
                si = ins.sync_info
                if (si is not None and si.on_wait
                        and len(si.on_wait) > budget
                        and ins.opcode not in ("EventSemaphore",)):
                    waits = list(si.on_wait)
                    excess, keep = waits[:-budget], waits[-budget:]
                    for i, wv in enumerate(excess):
                        nop = mybir.InstNoOp(
                            name=f"{ins.name}-sw{i}", engine=ins.engine,
                            bass_nofuse=True,
                            sync_info=mybir.SyncInfo(on_wait=[wv], on_update=[]))
                        out.append(nop)
                    si.on_wait = keep
                out.append(ins)
            blk.instructions[:] = out


def _wrap16(flat):
    """dma_gather index layout: [128, n/16] int16; index j sits at
    [16*g + j%16, j//16], replicated for all 8 groups g."""
    n = flat.shape[0]
    assert n % 16 == 0
    base = flat.reshape(n // 16, 16).T.astype(np.int16)
    return np.tile(base, (8, 1))


def _iota_full():
    return np.tile(np.arange(P, dtype=NP_BF16)[None, :], (P, 1))


# =========================================================================
# host-side preprocessing
# =========================================================================

class Meta:
    pass


def preprocess(x_in, edge_index, batch, W1, b1, W2, b2, n_cores=8):
    pr = Meta()
    N, IN = x_in.shape
    D = W1.shape[1]
    K = W2.shape[1]

    src = np.ascontiguousarray(edge_index[0]).astype(np.int64)
    dst = np.ascontiguousarray(edge_index[1]).astype(np.int64)
    batch = np.asarray(batch).astype(np.int64)

    WPC = int(np.ceil(N / n_cores / P))
    NS = WPC * P
    Npad = NS * n_cores
    NT = Npad // P
    assert Npad % PARTS == 0
    PS = Npad // PARTS
    PSZ = PS + P
    TPG = PS // P
    TB = None
    for cand in (7, 8, 4, 2, 1):
        if TPG % cand == 0:
            TB = cand
            break
    pr.TB = TB

    deg = np.bincount(dst, minlength=N).astype(np.float64)
    dinv_pad = np.ones(Npad, np.float32)
    dinv_pad[:N] = (1.0 / np.sqrt(deg + 1.0)).astype(np.float32)

    pr.__dict__.update(dict(
        N=N, B=128, IN=IN, D=D, K=K, n_cores=n_cores, WPC=WPC, NS=NS,
        Npad=Npad, NT=NT, PS=PS, PSZ=PSZ, dinv_pad=dinv_pad,
        W1=W1.astype(np.float32), b1=b1.astype(np.float32),
        W2=W2.astype(np.float32), b2=b2.astype(np.float32),
    ))

    xs_pad = np.zeros((Npad, IN), np.float32)
    xs_pad[:N] = x_in * dinv_pad[:N, None]

    # ---- per-core permutation + edge slotting by (dst window, table part)
    pr.perm, pr.xT_A, pr.dinvT = [], [], []
    core_srcloc, core_key, core_dstloc, core_cnt = [], [], [], []
    maxcnt = 0
    for c in range(n_cores):
        shard = np.arange(c * NS, (c + 1) * NS)
        others = np.concatenate(
            [np.arange(0, c * NS), np.arange((c + 1) * NS, Npad)])
        perm = np.concatenate([shard, others])
        rowpos = np.empty(Npad, np.int64)
        rowpos[perm] = np.arange(Npad)
        pr.perm.append(perm)
        pr.xT_A.append(np.ascontiguousarray(xs_pad[perm].T).astype(NP_BF16))
        pr.dinvT.append(np.ascontiguousarray(
            dinv_pad[shard].reshape(WPC, P).T))

        sel = (dst >= c * NS) & (dst < (c + 1) * NS)
        es, ed = src[sel], dst[sel]
        erow = rowpos[es]
        part = erow // PS
        wloc = (ed - c * NS) // P
        key = wloc * PARTS + part
        order = np.argsort(key, kind="stable")
        cnt = np.bincount(key, minlength=WPC * PARTS)
        maxcnt = max(maxcnt, int(cnt.max()))
        core_srcloc.append((erow % PS)[order])
        core_dstloc.append(
            (ed[order] - c * NS - wloc[order] * P).astype(np.float32))
        core_key.append(key[order])
        core_cnt.append(cnt)

    C4 = max(1, int(np.ceil(maxcnt / P)))
    SL = C4 * P
    pr.C4 = C4
    NCHUNK = WPC * PARTS * C4

    pr.srcg16, pr.dstlocT = [], []
    for c in range(n_cores):
        key, cnt = core_key[c], core_cnt[c]
        starts = np.zeros(WPC * PARTS + 1, np.int64)
        np.cumsum(cnt, out=starts[1:])
        slots = np.full((WPC * PARTS, SL), PS, np.int64)
        dloc = np.full((WPC * PARTS, SL), -1.0, np.float32)
        pos = np.arange(len(key)) - starts[key]
        slots[key, pos] = core_srcloc[c]
        dloc[key, pos] = core_dstloc[c]
        pr.srcg16.append(_wrap16(slots.reshape(-1)))
        pr.dstlocT.append(np.ascontiguousarray(
            dloc.reshape(WPC * PARTS, C4, P)
                .transpose(2, 0, 1).reshape(P, NCHUNK)).astype(NP_BF16))

    # ---- pooling prep
    GW, NPW = 4, 8
    pr.GW, pr.NPW = GW, NPW
    pr.gbase = []
    core_win_nodes = []
    tpw = 1
    for c in range(n_cores):
        lo, hi = c * NS, min((c + 1) * NS, N)
        if lo >= N:
            pr.gbase.append(0)
            core_win_nodes.append([np.empty(0, np.int64)] * NPW)
            continue
        gb = int(batch[lo])
        assert int(batch[hi - 1]) - gb + 1 <= NPW * GW
        pr.gbase.append(gb)
        nodes = np.arange(lo, hi)
        gl = batch[lo:hi] - gb
        wins = []
        for w2 in range(NPW):
            sel = nodes[(gl >= w2 * GW) & (gl < (w2 + 1) * GW)]
            wins.append(sel)
            tpw = max(tpw, int(np.ceil(len(sel) / P)))
        core_win_nodes.append(wins)
    pr.TPW, pr.PT = tpw, NPW * tpw

    pr.poolidx16, pr.batchlocT, pr.poolnode = [], [], []
    for c in range(n_cores):
        pidx = np.zeros(pr.PT * P, np.int64)
        bloc = np.full((P, pr.PT), -1.0, np.float32)
        pnode = np.full(pr.PT * P, -1, np.int64)
        for w2 in range(NPW):
            sel = core_win_nodes[c][w2]
            for t in range(tpw):
                tt = w2 * tpw + t
                seg = sel[t * P:(t + 1) * P]
                n = len(seg)
                if n:
                    pidx[tt * P:tt * P + n] = seg - c * NS
                    bloc[:n, tt] = (batch[seg] - pr.gbase[c] - w2 * GW)
                    pnode[tt * P:tt * P + n] = seg
        pr.poolidx16.append(_wrap16(pidx))
        pr.batchlocT.append(bloc.astype(NP_BF16))
        pr.poolnode.append(pnode)

    # ---- const bundles (layout shared across cores)
    pr.cbA = ConstBundle()
    pr.cbA.add("dinvT", np.float32, WPC)
    pr.cbA.add("srcg", np.int16, NCHUNK * 8)
    pr.cbA.add("bt", np.float32, D)
    pr.cbA.add("w1", NP_BF16, D)
    pr.cbA.add("iota", NP_BF16, P)
    pr.cbA.add("dstloc", NP_BF16, NCHUNK)

    pr.cbB = ConstBundle()
    pr.cbB.add("dinvT", np.float32, WPC)
    pr.cbB.add("srcg", np.int16, NCHUNK * 8)
    pr.cbB.add("bt", np.float32, K)
    pr.cbB.add("poolidx", np.int16, pr.PT * 8)
    pr.cbB.add("w2", NP_BF16, K)
    pr.cbB.add("iota", NP_BF16, P)
    pr.cbB.add("dstloc", NP_BF16, NCHUNK)
    pr.cbB.add("bloc", NP_BF16, pr.PT)
    return pr


# =========================================================================
# Bass program builders
# =========================================================================

def _build_table(nc, pools, pr, xT_d, w_sb, gtab, g_shard, FIN, FOUT, FPAD):
    """h-table build: per part, TB-tile groups; f32 rows -> gtab + zero rows."""
    TB, WPC, PS = pr.TB, pr.WPC, pr.PS
    TPG = PS // P
    writes = []
    xtp, pp8, g8p = pools["xt"], pools["ps8"], pools["g8"]
    for q in range(PARTS):
        for gi in range(TPG // TB):
            t0 = q * TPG + gi * TB
            xt = xtp.tile([FIN, TB * P], BF16)
            nc.sync.dma_start(out=xt[:], in_=xT_d[:, t0 * P:(t0 + TB) * P])
            ps = pp8.tile([P, TB * FOUT], F32)
            for j in range(TB):
                nc.tensor.matmul(ps[:, j * FOUT:(j + 1) * FOUT],
                                 lhsT=xt[:, j * P:(j + 1) * P],
                                 rhs=w_sb[:], start=True, stop=True)
            g8 = g8p.tile([P, TB * FPAD], F32)
            if FPAD == FOUT:
                nc.scalar.copy(out=g8[:], in_=ps[:])
            else:
                nc.scalar.copy(
                    out=g8[:].rearrange("p (t d) -> p t d", d=FPAD)[:, :, :FOUT],
                    in_=ps[:].rearrange("p (t d) -> p t d", d=FOUT))
                nc.scalar.activation(
                    out=g8[:].rearrange("p (t d) -> p t d", d=FPAD)[:, :, FOUT:],
                    in_=ps[:].rearrange("p (t d) -> p t d", d=FOUT)
                        [:, :, :FPAD - FOUT],
                    func=ActFn.Copy, scale=0.0)
            lo = t0
            if lo < WPC:
                nj = min(WPC - lo, TB)
                nc.vector.tensor_copy(
                    out=g_shard[:, lo * FOUT:(lo + nj) * FOUT],
                    in_=ps[:, :nj * FOUT])
            w = nc.sync.dma_start(
                out=gtab[q * pr.PSZ + gi * TB * P:
                         q * pr.PSZ + (gi + 1) * TB * P, :]
                    .rearrange("(t p) d -> p t d", p=P),
                in_=g8[:].rearrange("p (t d) -> p t d", d=FPAD))
            writes.append(w)
    zf = pools["const"].tile([P, FPAD], F32, name="zf_sb", tag="zf_sb")
    nc.vector.memset(zf[:], 0)
    for q in range(PARTS):
        w = nc.sync.dma_start(out=gtab[q * pr.PSZ + PS:(q + 1) * pr.PSZ, :],
                              in_=zf[:])
        writes.append(w)
    return writes


def _edge_phase(nc, pools, pr, gtab, srcg_sb, dstloc_sb, iota_sb,
                table_writes, FPAD, FUSE, finish):
    """Per dst-window: PARTS dma_gathers + M-matmul segment sum."""
    WPC, C4 = pr.WPC, pr.C4
    msp, mqp, mtp, pp = pools["msgs"], pools["msq"], pools["mt"], pools["ps"]
    first = [True]
    for w in range(WPC):
        mt = mtp.tile([P, PARTS * C4 * P], BF16)
        nc.vector.tensor_tensor(
            out=mt[:].rearrange("p (k r) -> p k r", r=P),
            in0=dstloc_sb[:, w * PARTS * C4:(w + 1) * PARTS * C4]
                .unsqueeze(2).to_broadcast([P, PARTS * C4, P]),
            in1=iota_sb[:].unsqueeze(1).to_broadcast([P, PARTS * C4, P]),
            op=AluOp.is_equal)
        ps = pp.tile([P, FUSE], F32)
        GCAP = 8   # chunks per gather instruction (= 64 descs per engine, HW max)
        for q in range(PARTS):
            for k0 in range(0, C4, GCAP):
                nk = min(GCAP, C4 - k0)
                cc0 = (w * PARTS + q) * C4 + k0
                msgs = msp.tile([P, GCAP * FPAD], F32)
                g = nc.gpsimd.dma_gather(
                    msgs[:, :nk * FPAD].rearrange("p (c e) -> p c e", e=FPAD),
                    gtab[q * pr.PSZ:(q + 1) * pr.PSZ, :],
                    srcg_sb[:, cc0 * 8:(cc0 + nk) * 8],
                    nk * P, nk * P, FPAD)
                if first[0]:
                    first[0] = False
                    for tw in table_writes:
                        add_dep_helper(g.ins, tw.ins, sync=True,
                                       reason="gather after table")
                msq = mqp.tile([P, GCAP * FUSE], BF16)
                if FUSE == FPAD:
                    nc.scalar.copy(out=msq[:, :nk * FUSE], in_=msgs[:, :nk * FPAD])
                else:
                    nc.scalar.copy(
                        out=msq[:, :nk * FUSE].rearrange("p (c e) -> p c e", e=FUSE),
                        in_=msgs[:, :nk * FPAD].rearrange("p (c e) -> p c e", e=FPAD)
                            [:, :, :FUSE])
                for k in range(nk):
                    nc.tensor.matmul(
                        ps[:],
                        lhsT=mt[:, (q * C4 + k0 + k) * P:(q * C4 + k0 + k + 1) * P],
                        rhs=msq[:, k * FUSE:(k + 1) * FUSE],
                        start=(q == 0 and k0 == 0 and k == 0),
                        stop=(q == PARTS - 1 and k0 + k == C4 - 1))
        finish(w, ps)


def _mk_pools(tc, es, extra=()):
    pools = {}
    names = [("const", 1, None), ("xt", 3, None), ("g8", 4, None),
             ("msgs", 6, None), ("msq", 6, None), ("mt", 3, None),
             ("xw", 4, None), ("ps8", 2, "PSUM"), ("ps", 2, "PSUM")]
    names += list(extra)
    for nm, bufs, space in names:
        kw = dict(name=nm, bufs=bufs)
        if space:
            kw["space"] = space
        pools[nm] = es.enter_context(tc.tile_pool(**kw))
    return pools


def build_A(pr, split=True):
    from contextlib import ExitStack
    IN, D, WPC, Npad, NS = pr.IN, pr.D, pr.WPC, pr.Npad, pr.NS
    CBW = pr.cbA.nbytes // 4

    nc = bacc.Bacc("TRN2")
    xT_d = nc.declare_dram_parameter("xT", [IN, Npad], BF16, isOutput=False)
    cb_d = nc.declare_dram_parameter("cb", [P, CBW], I32, isOutput=False)
    xout_d = nc.declare_dram_parameter("xout", [NS, D], BF16, isOutput=True)
    xsout_d = nc.declare_dram_parameter("xsout", [NS, D], BF16, isOutput=True)
    gtab = nc.dram_tensor("gtab", [PARTS * pr.PSZ, D], F32)

    with tile.TileContext(nc) as tc, ExitStack() as es:
        pools = _mk_pools(tc, es)
        cp = pools["const"]
        cb_sb = cp.tile([P, CBW], I32, name="cb_sb", tag="cb_sb")
        nc.sync.dma_start(out=cb_sb[:], in_=cb_d[:])
        nc.vector.tensor_copy(out=cb_sb[:], in_=cb_sb[:])
        V = lambda name: pr.cbA.view(cb_sb, name)
        g_shard = cp.tile([P, WPC * D], F32)

        table_writes = _build_table(nc, pools, pr, xT_d, V("w1"), gtab,
                                    g_shard, IN, D, D)

        dinvT_sb, bt_sb, iota_sb = V("dinvT"), V("bt"), V("iota")
        xwp = pools["xw"]

        def finish(w, ps):
            t1 = xwp.tile([P, D], F32, tag="t1")
            nc.vector.tensor_tensor(out=t1[:], in0=ps[:],
                                    in1=g_shard[:, w * D:(w + 1) * D],
                                    op=AluOp.add)
            xf = xwp.tile([P, D], F32, tag="xf")
            nc.vector.tensor_scalar(
                out=xf[:], in0=t1[:], scalar1=dinvT_sb[:, w:w + 1],
                scalar2=None, op0=AluOp.mult)
            xq = xwp.tile([P, D], BF16, tag="xq")
            nc.vector.tensor_tensor(out=xq[:], in0=xf[:], in1=bt_sb[:],
                                    op=AluOp.add)
            xsq = xwp.tile([P, D], BF16, tag="xsq")
            nc.vector.tensor_scalar(
                out=xsq[:], in0=xq[:], scalar1=dinvT_sb[:, w:w + 1],
                scalar2=None, op0=AluOp.mult)
            nc.sync.dma_start(out=xout_d[w * P:(w + 1) * P, :], in_=xq[:])
            nc.sync.dma_start(out=xsout_d[w * P:(w + 1) * P, :], in_=xsq[:])

        _edge_phase(nc, pools, pr, gtab, V("srcg"), V("dstloc"), iota_sb,
                    table_writes, D, D, finish)
    nc.compile()
    if split:
        _split_waits(nc)
    return nc


def build_B(pr, split=True):
    from contextlib import ExitStack
    D, K, WPC, Npad, NS = pr.D, pr.K, pr.WPC, pr.Npad, pr.NS
    GW, NPW, TPW, PT = pr.GW, pr.NPW, pr.TPW, pr.PT
    CBW = pr.cbB.nbytes // 4

    nc = bacc.Bacc("TRN2")
    xT2_d = nc.declare_dram_parameter("xT2", [D, Npad], BF16, isOutput=False)
    cb_d = nc.declare_dram_parameter("cb", [P, CBW], I32, isOutput=False)
    xpool_d = nc.declare_dram_parameter("xpool", [PT * P, D], BF16, isOutput=False)
    pool_d = nc.declare_dram_parameter("pool", [P, NPW * D], F32, isOutput=True)
    gtab = nc.dram_tensor("g2tab", [PARTS * pr.PSZ, D], F32)
    s_hbm = nc.dram_tensor("s_hbm", [NS + P, D], F32)

    with tile.TileContext(nc) as tc, ExitStack() as es:
        pools = _mk_pools(tc, es, extra=[
            ("sw", 4, None), ("xp", 3, None), ("spl", 2, None),
            ("plp", 2, "PSUM")])
        cp = pools["const"]
        cb_sb = cp.tile([P, CBW], I32, name="cb_sb", tag="cb_sb")
        nc.sync.dma_start(out=cb_sb[:], in_=cb_d[:])
        nc.vector.tensor_copy(out=cb_sb[:], in_=cb_sb[:])
        V = lambda name: pr.cbB.view(cb_sb, name)
        g_shard = cp.tile([P, WPC * K], F32)

        table_writes = _build_table(nc, pools, pr, xT2_d, V("w2")[:D, :],
                                    gtab, g_shard, D, K, D)

        dinvT_sb, bt_sb, iota_sb = V("dinvT"), V("bt"), V("iota")
        poolidx_sb, bloc_sb = V("poolidx"), V("bloc")
        swp = pools["sw"]

        s_sb = cp.tile([P, WPC * D], F32)
        nc.vector.memset(s_sb[:], 0)

        def finish(w, ps):
            t1 = swp.tile([P, K], F32, tag="t1")
            nc.vector.tensor_tensor(out=t1[:], in0=ps[:],
                                    in1=g_shard[:, w * K:(w + 1) * K],
                                    op=AluOp.add)
            sl = swp.tile([P, K], F32, tag="sl")
            nc.vector.tensor_scalar(
                out=sl[:], in0=t1[:], scalar1=dinvT_sb[:, w:w + 1],
                scalar2=None, op0=AluOp.mult)
            sl2 = swp.tile([P, K], F32, tag="sl2")
            nc.vector.tensor_tensor(out=sl2[:], in0=sl[:], in1=bt_sb[:],
                                    op=AluOp.add)
            ex = swp.tile([P, K], F32, tag="ex")
            nc.scalar.activation(out=ex[:], in_=sl2[:], func=ActFn.Exp)
            sm = swp.tile([P, 1], F32, tag="sm")
            nc.vector.tensor_reduce(out=sm[:], in_=ex[:],
                                    axis=mybir.AxisListType.X, op=AluOp.add)
            rc = swp.tile([P, 1], F32, tag="rc")
            nc.vector.reciprocal(out=rc[:], in_=sm[:])
            nc.vector.tensor_scalar(
                out=s_sb[:, w * D:w * D + K], in0=ex[:],
                scalar1=rc[:, :1], scalar2=None, op0=AluOp.mult)

        _edge_phase(nc, pools, pr, gtab, V("srcg"), V("dstloc"), iota_sb,
                    table_writes, D, K, finish)

        # ---- pooling
        zs = cp.tile([P, D], F32, name="zs_sb", tag="zs_sb")
        nc.vector.memset(zs[:], 0)
        s_write2 = nc.sync.dma_start(out=s_hbm[NS:NS + P, :], in_=zs[:])
        s_write = nc.sync.dma_start(
            out=s_hbm[:NS, :].rearrange("(w p) k -> p w k", p=P),
            in_=s_sb[:].rearrange("p (w k) -> p w k", k=D))
        pool_sb = cp.tile([P, NPW * D], F32)
        splp, xpp, mtp, plp = (pools["spl"], pools["xp"], pools["mt"],
                               pools["plp"])
        for w2 in range(NPW):
            spool = splp.tile([P, TPW * D], F32, tag="spool")
            for t0 in range(0, TPW, 4):
                nt = min(4, TPW - t0)
                gp = nc.gpsimd.dma_gather(
                    spool[:, t0 * D:(t0 + nt) * D]
                        .rearrange("p (c e) -> p c e", e=D),
                    s_hbm[:, :],
                    poolidx_sb[:, (w2 * TPW + t0) * 8:(w2 * TPW + t0 + nt) * 8],
                    nt * P, nt * P, D)
                add_dep_helper(gp.ins, s_write.ins, sync=True,
                               reason="pool gather after S write")
                add_dep_helper(gp.ins, s_write2.ins, sync=True,
                               reason="pool gather after S pad write")
            spq = splp.tile([P, TPW * K], BF16, tag="spq")
            nc.scalar.copy(
                out=spq[:].rearrange("p (c e) -> p c e", e=K),
                in_=spool[:].rearrange("p (c e) -> p c e", e=D)[:, :, :K])
            pps = plp.tile([P, D], F32)
            for t in range(TPW):
                tt = w2 * TPW + t
                xp = xpp.tile([P, D], BF16)
                nc.sync.dma_start(out=xp[:], in_=xpool_d[tt * P:(tt + 1) * P, :])
                mk = mtp.tile([P, GW], BF16, tag="mk")
                nc.vector.tensor_tensor(
                    out=mk[:], in0=bloc_sb[:, tt:tt + 1].to_broadcast([P, GW]),
                    in1=iota_sb[:, :GW], op=AluOp.is_equal)
                sst = mtp.tile([P, GW * K], BF16, tag="sst")
                nc.vector.tensor_tensor(
                    out=sst[:].rearrange("p (g k) -> p g k", k=K),
                    in0=spq[:, t * K:(t + 1) * K]
                        .unsqueeze(1).to_broadcast([P, GW, K]),
                    in1=mk[:].unsqueeze(2).to_broadcast([P, GW, K]),
                    op=AluOp.mult)
                nc.tensor.matmul(pps[:], lhsT=sst[:], rhs=xp[:],
                                 start=(t == 0), stop=(t == TPW - 1))
            nc.vector.tensor_copy(out=pool_sb[:, w2 * D:(w2 + 1) * D],
                                  in_=pps[:])
        nc.sync.dma_start(out=pool_d[:], in_=pool_sb[:])
    nc.compile()
    if split:
        _split_waits(nc)
    return nc


# =========================================================================
# runners + glue
# =========================================================================

_TRACE = bool(int(os.environ.get("KERNEL_TRACE", "0")))
_LAST_EXEC_NS = {}
_LAST_WALL = {}


def _run_spmd(nc, in_maps, tag):
    import time
    core_ids = list(range(len(in_maps)))
    t0 = time.time()
    res = run_bass_kernel_spmd(nc, in_maps, core_ids, trace=_TRACE)
    _LAST_WALL[tag] = time.time() - t0
    if res.exec_time_ns is not None:
        _LAST_EXEC_NS[tag] = res.exec_time_ns
    return res.results


def make_in_maps_A(pr):
    maps = []
    for c in range(pr.n_cores):
        cb = pr.cbA.pack(dict(
            dinvT=pr.dinvT[c], srcg=pr.srcg16[c],
            bt=np.tile(pr.b1[None, :], (P, 1)).astype(np.float32),
            w1=np.tile(pr.W1.astype(NP_BF16), (1, 1)), iota=_iota_full(),
            dstloc=pr.dstlocT[c]))
        maps.append(dict(xT=pr.xT_A[c], cb=cb))
    return maps


def make_in_maps_B(pr, x_q, xs_q):
    D, K = pr.D, pr.K
    w2pad = np.zeros((P, K), NP_BF16)
    w2pad[:D] = pr.W2.astype(NP_BF16)
    x_f = x_q.astype(np.float32)
    maps = []
    for c in range(pr.n_cores):
        cb = pr.cbB.pack(dict(
            dinvT=pr.dinvT[c], srcg=pr.srcg16[c],
            bt=np.tile(pr.b2[None, :], (P, 1)).astype(np.float32),
            poolidx=pr.poolidx16[c], w2=w2pad, iota=_iota_full(),
            dstloc=pr.dstlocT[c], bloc=pr.batchlocT[c]))
        pn = pr.poolnode[c]
        xp = np.zeros((pr.PT * P, D), np.float32)
        v = pn >= 0
        xp[v] = x_f[pn[v]]
        maps.append(dict(
            xT2=np.ascontiguousarray(xs_q[pr.perm[c]].T), cb=cb,
            xpool=xp.astype(NP_BF16)))
    return maps


def reduce_pool(pr, pool_outs):
    D, K = pr.D, pr.K
    pooled = np.zeros((pr.B, K, D), np.float64)
    for c in range(pr.n_cores):
        po = np.asarray(pool_outs[c]).astype(np.float64)
        gb = pr.gbase[c]
        blk = po.reshape(pr.GW, K, pr.NPW, D)
        for g_loc in range(pr.GW):
            for w2 in range(pr.NPW):
                g = gb + w2 * pr.GW + g_loc
                if g < pr.B:
                    pooled[g] += blk[g_loc, :, w2, :]
    return pooled.astype(np.float32)


def kernel(x_in, edge_index, batch, W1, b1, W2, b2):
    n_cores = 8
    pr = preprocess(x_in, edge_index, batch, W1, b1, W2, b2, n_cores)

    ncA = build_A(pr)
    resA = _run_spmd(ncA, make_in_maps_A(pr), "A")
    x_q = np.vstack([resA[c]["xout"] for c in range(n_cores)])
    xs_q = np.vstack([resA[c]["xsout"] for c in range(n_cores)])

    ncB = build_B(pr)
    resB = _run_spmd(ncB, make_in_maps_B(pr, x_q, xs_q), "B")
    return reduce_pool(pr, [resB[c]["pool"] for c in range(n_cores)])



# revision 7
# speedup vs baseline: 39.2281x; 39.2281x over previous
"""DiffPool-like GNN (two GCN convs + softmax clustering + weighted pooling)
as ONE fused Bass/Tile SPMD launch on 8 Trainium2 NeuronCores.

Distribution (matches the sharding hint):
  * nodes partitioned into 8 contiguous shards; each core owns the edges whose
    dst falls in its shard (host buckets edges by 128-node dst window);
  * W1/W2 replicated (const bundle);
  * each core computes h = (D^-1/2 x) @ W rows for ITS OWN shard only, then an
    on-device AllGather assembles the full [Npad, 64] f32 feature table in
    natural node order (the "halo exchange" - here a full gather since edges
    are random);
  * per-edge messages fetched with the MoE dma_gather primitive (256B rows,
    int16 indices -> the table is addressed in 4 parts of Npad/4 rows; padding
    slots point at row 0 and are masked by the one-hot matmul);
  * segment-sum on the tensor engine: per 128-slot chunk, a one-hot matrix
    M[p, r] = (dstloc[p] == r) is built on the vector engine and
    agg += M.T @ msgs accumulates in PSUM across the window's chunks;
  * conv output x1 = dinv*(agg + g_self) + b stays resident in SBUF; the
    conv2 table rows xs1 @ W2 are produced per-window (transpose via an
    identity matmul) and AllGathered the same way;
  * pooling without any gather: per window, onehotB[n, g] = (batch[n] == g)
    over all B=128 graphs and an outer product S[n,k]*x1[n,j] feed
    pooled[g, k*64+j] += onehotB.T @ outer, accumulated in PSUM across all
    windows; a ReduceScatter leaves each core with 16 graph rows ("all-reduce
    the per-(graph,cluster) pooled partial sums");
  * host work: reshape the concatenated ReduceScatter output.

The walrus build in this container encodes at most ONE sync wait per
instruction; _split_waits() rewrites the scheduled BIR, moving excess waits
onto injected single-wait NoOps.
"""

import os
import sys
import numpy as np

sys.path.insert(0, "/opt/trn_rl_repo")

import ml_dtypes  # noqa: E402
import concourse.bacc as bacc  # noqa: E402
import concourse.mybir as mybir  # noqa: E402
import concourse.tile as tile  # noqa: E402
from concourse.tile_rust import add_dep_helper  # noqa: E402

P = 128
BF16 = mybir.dt.bfloat16
F32 = mybir.dt.float32
I16 = mybir.dt.int16
I32 = mybir.dt.int32
NP_BF16 = ml_dtypes.bfloat16

AluOp = mybir.AluOpType
ActFn = mybir.ActivationFunctionType

_DT_MAP = {
    np.dtype(np.float32): F32,
    np.dtype(np.int16): I16,
    np.dtype(NP_BF16): BF16,
}

PARTS = 4
GCAP = 8  # chunks per gather instruction (= 64 descs per engine, HW max)


class ConstBundle:
    """Packs [128, n] arrays of mixed dtypes into one [128, W] int32 array."""

    def __init__(self):
        self.fields = {}
        self.nbytes = 0

    def add(self, name, dtype, n):
        dt = np.dtype(dtype)
        b = dt.itemsize * n
        b4 = (b + 3) & ~3
        self.fields[name] = (self.nbytes, dt, n)
        self.nbytes += b4

    def pack(self, arrays):
        w = self.nbytes // 4
        out = np.zeros((P, w), np.int32)
        ob = out.view(np.uint8)
        for name, (off, dt, n) in self.fields.items():
            a = np.ascontiguousarray(arrays[name])
            assert a.dtype == dt and a.shape == (P, n), (name, a.dtype, a.shape)
            ob[:, off:off + dt.itemsize * n] = a.view(np.uint8)
        return out

    def view(self, cb_sb, name):
        off, dt, n = self.fields[name]
        b4 = (dt.itemsize * n + 3) & ~3
        v = cb_sb[:, off // 4:(off + b4) // 4].bitcast(_DT_MAP[dt])
        return v[:, :n]


def _split_waits(nc, budget=1):
    """Move excess sync waits onto injected single-wait same-engine NoOps.
    The walrus in this container encodes at most one wait per instruction."""
    for fn in nc.m.functions:
        for blk in fn.blocks:
            out = []
            for ins in blk.instructions:
                si = ins.sync_info
                if (si is not None and si.on_wait
                        and len(si.on_wait) > budget
                        and ins.opcode not in ("EventSemaphore",)):
                    waits = list(si.on_wait)
                    excess, keep = waits[:-budget], waits[-budget:]
                    for i, wv in enumerate(excess):
                        nop = mybir.InstNoOp(
                            name=f"{ins.name}-sw{i}", engine=ins.engine,
                            bass_nofuse=True,
                            sync_info=mybir.SyncInfo(on_wait=[wv], on_update=[]))
                        out.append(nop)
                    si.on_wait = keep
                out.append(ins)
            blk.instructions[:] = out


def _wrap16_base(flat):
    """dma_gather index layout base: [16, n/16] int16; index j sits at
    [j%16, j//16]. The device replicates it to all 8 groups (128 rows)."""
    n = flat.shape[0]
    assert n % 16 == 0
    return np.ascontiguousarray(flat.reshape(n // 16, 16).T.astype(np.int16))


def _iota_full():
    return np.tile(np.arange(P, dtype=NP_BF16)[None, :], (P, 1))


# =========================================================================
# host-side preprocessing
# =========================================================================

class Meta:
    pass


def preprocess(x_in, edge_index, batch, W1, b1, W2, b2, n_cores=8):
    pr = Meta()
    N, IN = x_in.shape
    D = W1.shape[1]
    K = W2.shape[1]
    assert IN == P

    src = np.ascontiguousarray(edge_index[0]).astype(np.int64)
    dst = np.ascontiguousarray(edge_index[1]).astype(np.int64)
    batch = np.asarray(batch).astype(np.int64)

    WPC = int(np.ceil(N / n_cores / P))
    NS = WPC * P
    Npad = NS * n_cores
    assert Npad % PARTS == 0
    PS = Npad // PARTS
    assert PS < 2 ** 15

    deg = np.bincount(dst, minlength=N).astype(np.float64)
    dinv_pad = np.ones(Npad, np.float32)
    dinv_pad[:N] = (1.0 / np.sqrt(deg + 1.0)).astype(np.float32)

    pr.__dict__.update(dict(
        N=N, B=P, IN=IN, D=D, K=K, n_cores=n_cores, WPC=WPC, NS=NS,
        Npad=Npad, PS=PS,
        W1=W1.astype(np.float32), b1=b1.astype(np.float32),
        W2=W2.astype(np.float32), b2=b2.astype(np.float32),
    ))

    # ---- per-core xs^T shard (xs = x * dinv), bf16 [IN, NS]
    xs = (x_in * dinv_pad[:N, None]).astype(NP_BF16)
    pr.xT = []
    for c in range(n_cores):
        lo, hi = c * NS, min((c + 1) * NS, N)
        blk = np.zeros((IN, NS), NP_BF16)
        blk[:, :hi - lo] = xs[lo:hi].T
        pr.xT.append(blk)

    pr.dinvT = [np.ascontiguousarray(
        dinv_pad[c * NS:(c + 1) * NS].reshape(WPC, P).T)
        for c in range(n_cores)]

    # batch (global graph id 0..127) per shard slot, -1 for pad rows
    bloc_pad = np.full(Npad, -1.0, np.float32)
    bloc_pad[:N] = batch.astype(np.float32)
    pr.batchlocT = [np.ascontiguousarray(
        bloc_pad[c * NS:(c + 1) * NS].reshape(WPC, P).T).astype(NP_BF16)
        for c in range(n_cores)]

    # ---- global edge slotting by (core, dst window, table part)
    core = dst // NS
    wloc = (dst - core * NS) // P
    part = src // PS
    key = ((core * WPC + wloc) * PARTS + part)
    order = np.argsort(key, kind="stable")
    key_o = key[order]
    cnt = np.bincount(key, minlength=n_cores * WPC * PARTS)
    C4 = max(1, int(np.ceil(cnt.max() / P)))
    SL = C4 * P
    NCHUNK = WPC * PARTS * C4
    pr.C4, pr.NCHUNK = C4, NCHUNK

    starts = np.zeros(n_cores * WPC * PARTS + 1, np.int64)
    np.cumsum(cnt, out=starts[1:])
    slots = np.zeros((n_cores * WPC * PARTS, SL), np.int64)
    dloc = np.full((n_cores * WPC * PARTS, SL), -1.0, np.float32)
    pos = np.arange(len(key_o)) - starts[key_o]
    slots[key_o, pos] = src[order] % PS
    dloc[key_o, pos] = (dst[order] % P).astype(np.float32)

    pr.idx16, pr.dstlocT = [], []
    for c in range(n_cores):
        s = slots[c * WPC * PARTS:(c + 1) * WPC * PARTS]
        d = dloc[c * WPC * PARTS:(c + 1) * WPC * PARTS]
        pr.idx16.append(_wrap16_base(s.reshape(-1)))
        pr.dstlocT.append(np.ascontiguousarray(
            d.reshape(WPC * PARTS, C4, P)
             .transpose(2, 0, 1).reshape(P, NCHUNK)).astype(NP_BF16))

    # ---- const bundle (layout shared across cores)
    cb = ConstBundle()
    cb.add("dinvT", np.float32, WPC)
    cb.add("bt1", np.float32, D)
    cb.add("bt2", np.float32, K)
    cb.add("w1", NP_BF16, D)
    cb.add("w2", NP_BF16, K)
    cb.add("iota", NP_BF16, P)
    cb.add("piota", NP_BF16, 2)
    cb.add("bloc", NP_BF16, WPC)
    cb.add("dstloc", NP_BF16, NCHUNK)
    pr.cb = cb
    return pr


def make_in_maps(pr):
    D, K = pr.D, pr.K
    w2pad = np.zeros((P, K), NP_BF16)
    w2pad[:D] = pr.W2.astype(NP_BF16)
    piota = np.zeros((P, 2), NP_BF16)
    piota[:, 0] = np.arange(P, dtype=NP_BF16)
    maps = []
    for c in range(pr.n_cores):
        cb = pr.cb.pack(dict(
            dinvT=pr.dinvT[c],
            bt1=np.tile(pr.b1[None, :], (P, 1)).astype(np.float32),
            bt2=np.tile(pr.b2[None, :], (P, 1)).astype(np.float32),
            w1=pr.W1.astype(NP_BF16), w2=w2pad,
            iota=_iota_full(), piota=piota,
            bloc=pr.batchlocT[c], dstloc=pr.dstlocT[c]))
        maps.append(dict(xT=pr.xT[c], idx=pr.idx16[c], cb=cb))
    return maps


# =========================================================================
# Bass program builder (single fused launch)
# =========================================================================

def _edge_phase(nc, pools, pr, gtab, idx_sb, dstloc_sb, iota_sb,
                gather_dep, FPAD, FUSE, finish):
    """Per dst-window: PARTS dma_gathers + M-matmul segment sum."""
    WPC, C4, PS = pr.WPC, pr.C4, pr.PS
    msp, mqp, mtp, pp = pools["msgs"], pools["msq"], pools["mt"], pools["ps"]
    first = [True]
    for w in range(WPC):
        mt = mtp.tile([P, PARTS * C4 * P], BF16)
        nc.vector.tensor_tensor(
            out=mt[:].rearrange("p (k r) -> p k r", r=P),
            in0=dstloc_sb[:, w * PARTS * C4:(w + 1) * PARTS * C4]
                .unsqueeze(2).to_broadcast([P, PARTS * C4, P]),
            in1=iota_sb[:].unsqueeze(1).to_broadcast([P, PARTS * C4, P]),
            op=AluOp.is_equal)
        ps = pp.tile([P, FUSE], F32)
        for q in range(PARTS):
            for k0 in range(0, C4, GCAP):
                nk = min(GCAP, C4 - k0)
                cc0 = (w * PARTS + q) * C4 + k0
                msgs = msp.tile([P, GCAP * FPAD], F32)
                g = nc.gpsimd.dma_gather(
                    msgs[:, :nk * FPAD].rearrange("p (c e) -> p c e", e=FPAD),
                    gtab[q * PS:(q + 1) * PS, :],
                    idx_sb[:, cc0 * 8:(cc0 + nk) * 8],
                    nk * P, nk * P, FPAD)
                if first[0]:
                    first[0] = False
                    add_dep_helper(g.ins, gather_dep.ins, sync=True,
                                   reason="gather after table allgather")
                msq = mqp.tile([P, GCAP * FUSE], BF16)
                if FUSE == FPAD:
                    nc.scalar.copy(out=msq[:, :nk * FUSE],
                                   in_=msgs[:, :nk * FPAD])
                else:
                    nc.scalar.copy(
                        out=msq[:, :nk * FUSE]
                            .rearrange("p (c e) -> p c e", e=FUSE),
                        in_=msgs[:, :nk * FPAD]
                            .rearrange("p (c e) -> p c e", e=FPAD)[:, :, :FUSE])
                for k in range(nk):
                    nc.tensor.matmul(
                        ps[:],
                        lhsT=mt[:, (q * C4 + k0 + k) * P:
                                   (q * C4 + k0 + k + 1) * P],
                        rhs=msq[:, k * FUSE:(k + 1) * FUSE],
                        start=(q == 0 and k0 == 0 and k == 0),
                        stop=(q == PARTS - 1 and k0 + k == C4 - 1))
        finish(w, ps)


def build(pr, split=True):
    from contextlib import ExitStack
    IN, D, K, WPC, Npad, NS, PS = (pr.IN, pr.D, pr.K, pr.WPC, pr.Npad,
                                   pr.NS, pr.PS)
    NCHUNK, C4 = pr.NCHUNK, pr.C4
    CBW = pr.cb.nbytes // 4
    IDXW = NCHUNK * 8
    KD = K * D
    TB = 7
    assert WPC % TB == 0

    nc = bacc.Bacc("TRN2")
    xT_d = nc.declare_dram_parameter("xT", [IN, NS], BF16, isOutput=False)
    idx_d = nc.declare_dram_parameter("idx", [16, IDXW], I16, isOutput=False)
    cb_d = nc.declare_dram_parameter("cb", [P, CBW], I32, isOutput=False)
    pool_d = nc.declare_dram_parameter("pool", [P // 8, KD], F32,
                                       isOutput=True)

    gtab1 = nc.dram_tensor("gtab1", [Npad, D], F32)
    gtab2 = nc.dram_tensor("gtab2", [Npad, D], F32)
    ag1_in = nc.dram_tensor("ag1in", [NS, D], F32)
    ag2_in = nc.dram_tensor("ag2in", [NS, D], F32)
    rs_in = nc.dram_tensor("rsin", [P, KD], F32)
    rs_out = nc.dram_tensor("rsout", [P // 8, KD], F32)

    with tile.TileContext(nc) as tc, ExitStack() as es:
        pools = {}
        for nm, bufs, space in [
                ("const", 1, None), ("msgs", 4, None), ("msq", 4, None),
                ("mt", 2, None), ("xw", 4, None), ("hw", 3, None),
                ("ps", 2, "PSUM")]:
            kw = dict(name=nm, bufs=bufs)
            if space:
                kw["space"] = space
            pools[nm] = es.enter_context(tc.tile_pool(**kw))
        cp = pools["const"]

        cb_sb = cp.tile([P, CBW], I32, name="cb_sb", tag="cb_sb")
        nc.sync.dma_start(out=cb_sb[:], in_=cb_d[:])
        nc.vector.tensor_copy(out=cb_sb[:], in_=cb_sb[:])
        V = lambda name: pr.cb.view(cb_sb, name)
        dinvT_sb, iota_sb = V("dinvT"), V("iota")

        idx_sb = cp.tile([P, IDXW], I16, name="idx_sb", tag="idx_sb")
        for g in range(8):
            nc.sync.dma_start(out=idx_sb[g * 16:(g + 1) * 16, :], in_=idx_d[:])

        g1_shard = cp.tile([P, WPC * D], F32)
        g2_shard = cp.tile([P, WPC * K], F32)
        x1_sb = cp.tile([P, WPC * D], BF16)
        ident = cp.tile([P, P], BF16, name="ident", tag="ident")
        nc.vector.tensor_tensor(
            out=ident[:],
            in0=V("piota")[:, :1].to_broadcast([P, P]),
            in1=iota_sb[:], op=AluOp.is_equal)

        # ---- phase 1: own-shard h1 rows -> ag1_in; AllGather -> gtab1
        with tc.tile_pool(name="xt", bufs=1) as xtp, \
                tc.tile_pool(name="ps8", bufs=2, space="PSUM") as pp8:
            xt = xtp.tile([IN, NS], BF16)
            nc.sync.dma_start(out=xt[:], in_=xT_d[:])
            w1_sb = V("w1")
            writes = []
            for gi in range(WPC // TB):
                ps8 = pp8.tile([P, TB * D], F32)
                for j in range(TB):
                    t = gi * TB + j
                    nc.tensor.matmul(ps8[:, j * D:(j + 1) * D],
                                     lhsT=xt[:, t * P:(t + 1) * P],
                                     rhs=w1_sb[:], start=True, stop=True)
                nc.vector.tensor_copy(
                    out=g1_shard[:, gi * TB * D:(gi + 1) * TB * D],
                    in_=ps8[:])
                w = nc.sync.dma_start(
                    out=ag1_in[gi * TB * P:(gi + 1) * TB * P, :]
                        .rearrange("(t p) d -> p t d", p=P),
                    in_=g1_shard[:, gi * TB * D:(gi + 1) * TB * D]
                        .rearrange("p (t d) -> p t d", d=D))
                writes.append(w)
        ag1 = nc.gpsimd.collective_compute(
            "AllGather", AluOp.bypass,
            replica_groups=[list(range(pr.n_cores))],
            ins=[ag1_in[:, :]], outs=[gtab1[:, :]])
        for w in writes:
            add_dep_helper(ag1.ins, w.ins, sync=True, reason="ag1 after h1")

        # ---- phase 2: conv1 edge phase; finish computes x1 (kept in SBUF),
        #      xs1, and the conv2 table rows h2 = xs1 @ W2 -> ag2_in
        bt1_sb, w2_sb = V("bt1"), V("w2")
        xwp, hwp = pools["xw"], pools["hw"]
        es2 = ExitStack()
        pstp = es2.enter_context(tc.tile_pool(name="pst", bufs=2,
                                              space="PSUM"))
        ph2p = es2.enter_context(tc.tile_pool(name="ph2", bufs=2,
                                              space="PSUM"))
        h2_writes = []

        def finish1(w, ps):
            t1 = xwp.tile([P, D], F32, tag="t1")
            nc.vector.tensor_tensor(out=t1[:], in0=ps[:],
                                    in1=g1_shard[:, w * D:(w + 1) * D],
                                    op=AluOp.add)
            xf = xwp.tile([P, D], F32, tag="xf")
            nc.vector.tensor_scalar(
                out=xf[:], in0=t1[:], scalar1=dinvT_sb[:, w:w + 1],
                scalar2=None, op0=AluOp.mult)
            nc.vector.tensor_tensor(out=x1_sb[:, w * D:(w + 1) * D],
                                    in0=xf[:], in1=bt1_sb[:], op=AluOp.add)
            xsq = xwp.tile([P, D], BF16, tag="xsq")
            nc.vector.tensor_scalar(
                out=xsq[:], in0=x1_sb[:, w * D:(w + 1) * D],
                scalar1=dinvT_sb[:, w:w + 1], scalar2=None, op0=AluOp.mult)
            # transpose xs1 via identity matmul, then h2 = xs1 @ W2
            pst = pstp.tile([D, P], F32)
            nc.tensor.matmul(pst[:], lhsT=xsq[:], rhs=ident[:],
                             start=True, stop=True)
            xst = xwp.tile([D, P], BF16, tag="xst")
            nc.scalar.copy(out=xst[:], in_=pst[:])
            ph2 = ph2p.tile([P, K], F32)
            nc.tensor.matmul(ph2[:], lhsT=xst[:], rhs=w2_sb[:D, :],
                             start=True, stop=True)
            nc.vector.tensor_copy(out=g2_shard[:, w * K:(w + 1) * K],
                                  in_=ph2[:])
            h8 = hwp.tile([P, D], F32)
            nc.scalar.copy(out=h8[:, :K], in_=ph2[:])
            nc.scalar.activation(out=h8[:, K:], in_=ph2[:, :D - K],
                                 func=ActFn.Copy, scale=0.0)
            hw_ = nc.sync.dma_start(out=ag2_in[w * P:(w + 1) * P, :],
                                    in_=h8[:])
            h2_writes.append(hw_)

        _edge_phase(nc, pools, pr, gtab1, idx_sb, V("dstloc"), iota_sb,
                    ag1, D, D, finish1)
        es2.close()

        ag2 = nc.gpsimd.collective_compute(
            "AllGather", AluOp.bypass,
            replica_groups=[list(range(pr.n_cores))],
            ins=[ag2_in[:, :]], outs=[gtab2[:, :]])
        for w in h2_writes:
            add_dep_helper(ag2.ins, w.ins, sync=True, reason="ag2 after h2")

        # ---- phase 3: conv2 edge phase; finish computes softmax S and the
        #      pooled[g, k*D+j] += onehotB.T @ (S outer x1) accumulation
        bt2_sb, bloc_sb = V("bt2"), V("bloc")
        plq_pool = es.enter_context(tc.tile_pool(name="plq", bufs=1,
                                                 space="PSUM"))
        plq = [plq_pool.tile([P, 512], F32, name=f"plq{i}", tag=f"plq{i}")
               for i in range(4)]
        NQ = KD // 512

        def finish2(w, ps):
            t1 = xwp.tile([P, K], F32, tag="t1")
            nc.vector.tensor_tensor(out=t1[:], in0=ps[:],
                                    in1=g2_shard[:, w * K:(w + 1) * K],
                                    op=AluOp.add)
            sl = xwp.tile([P, K], F32, tag="xf")
            nc.vector.tensor_scalar(
                out=sl[:], in0=t1[:], scalar1=dinvT_sb[:, w:w + 1],
                scalar2=None, op0=AluOp.mult)
            sl2 = xwp.tile([P, K], F32, tag="sl2")
            nc.vector.tensor_tensor(out=sl2[:], in0=sl[:], in1=bt2_sb[:],
                                    op=AluOp.add)
            ex = xwp.tile([P, K], F32, tag="ex")
            nc.scalar.activation(out=ex[:], in_=sl2[:], func=ActFn.Exp)
            sm = xwp.tile([P, 1], F32, tag="sm")
            nc.vector.tensor_reduce(out=sm[:], in_=ex[:],
                                    axis=mybir.AxisListType.X, op=AluOp.add)
            rc = xwp.tile([P, 1], F32, tag="rc")
            nc.vector.reciprocal(out=rc[:], in_=sm[:])
            sq = xwp.tile([P, K], BF16, tag="sq")
            nc.vector.tensor_scalar(
                out=sq[:], in0=ex[:], scalar1=rc[:, :1], scalar2=None,
                op0=AluOp.mult)
            ob = xwp.tile([P, P], BF16, tag="ob")
            nc.vector.tensor_tensor(
                out=ob[:], in0=bloc_sb[:, w:w + 1].to_broadcast([P, P]),
                in1=iota_sb[:], op=AluOp.is_equal)
            outer = hwp.tile([P, KD], BF16)
            nc.vector.tensor_tensor(
                out=outer[:].rearrange("p (k j) -> p k j", j=D),
                in0=sq[:].unsqueeze(2).to_broadcast([P, K, D]),
                in1=x1_sb[:, w * D:(w + 1) * D]
                    .unsqueeze(1).to_broadcast([P, K, D]),
                op=AluOp.mult)
            for i in range(NQ):
                nc.tensor.matmul(plq[i][:], lhsT=ob[:],
                                 rhs=outer[:, i * 512:(i + 1) * 512],
                                 start=(w == 0), stop=(w == WPC - 1))

        _edge_phase(nc, pools, pr, gtab2, idx_sb, V("dstloc"), iota_sb,
                    ag2, D, K, finish2)

        # ---- pooling reduce-scatter -> output
        pool_sb = cp.tile([P, KD], F32, name="pool_sb", tag="pool_sb")
        for i in range(NQ):
            nc.vector.tensor_copy(out=pool_sb[:, i * 512:(i + 1) * 512],
                                  in_=plq[i][:])
        rw = nc.sync.dma_start(out=rs_in[:, :], in_=pool_sb[:])
        rs = nc.gpsimd.collective_compute(
            "ReduceScatter", AluOp.add,
            replica_groups=[list(range(pr.n_cores))],
            ins=[rs_in[:, :]], outs=[rs_out[:, :]])
        add_dep_helper(rs.ins, rw.ins, sync=True, reason="rs after pool")
        out_sb = cp.tile([P // 8, KD], F32, name="out_sb", tag="out_sb")
        rd = nc.sync.dma_start(out=out_sb[:], in_=rs_out[:, :])
        add_dep_helper(rd.ins, rs.ins, sync=True, reason="read after rs")
        nc.sync.dma_start(out=pool_d[:], in_=out_sb[:])
    nc.compile()
    if split:
        _split_waits(nc)
    return nc


# =========================================================================
# runner + glue
# =========================================================================

_EXEC_CACHE = {}


def exec_spmd(nc, in_maps):
    """Execute a prebuilt Bass module on len(in_maps) cores via PJRT.

    Mirrors concourse.bass2jax.run_bass_via_pjrt, but (a) caches the jitted
    callable per-module so repeated runs don't re-trace/re-compile XLA, and
    (b) fetches each output as ONE global [n_cores*rows, cols] array (one
    device round-trip) instead of per-core sliced fetches.
    Returns {name: global np.ndarray} with per-core rows concatenated.
    """
    import jax
    from jax.sharding import Mesh, PartitionSpec
    from jax.experimental.shard_map import shard_map
    from concourse import bass2jax, mybir as _mybir
    from concourse.bass2jax import (_bass_exec_p, install_neuronx_cc_hook,
                                    partition_id_tensor)

    n_cores = len(in_maps)
    key = id(nc)
    if key not in _EXEC_CACHE:
        install_neuronx_cc_hook()
        assert nc.dbg_addr is None or not nc.dbg_callbacks
        partition_name = (nc.partition_id_tensor.name
                          if nc.partition_id_tensor else None)
        in_names, out_names, out_avals, zero_outs = [], [], [], []
        for alloc in nc.m.functions[0].allocations:
            if not isinstance(alloc, _mybir.MemoryLocationSet):
                continue
            name = alloc.memorylocations[0].name
            if alloc.kind == "ExternalInput":
                if name != partition_name:
                    in_names.append(name)
            elif alloc.kind == "ExternalOutput":
                shape = tuple(alloc.tensor_shape)
                dtype = _mybir.dt.np(alloc.dtype)
                out_names.append(name)
                out_avals.append(jax.core.ShapedArray(shape, dtype))
                zero_outs.append(np.zeros(shape, dtype))
        n_params = len(in_names)
        all_in = list(in_names) + list(out_names)
        if partition_name is not None:
            all_in.append(partition_name)
        donate = tuple(range(n_params, n_params + len(out_avals)))

        def _body(*args):
            operands = list(args)
            if partition_name is not None:
                operands.append(partition_id_tensor())
            return tuple(_bass_exec_p.bind(
                *operands, out_avals=tuple(out_avals), in_names=tuple(all_in),
                out_names=tuple(out_names), lowering_input_output_aliases=(),
                sim_require_finite=True, sim_require_nnan=True, nc=nc))

        mesh = Mesh(np.asarray(jax.devices()[:n_cores]), ("core",))
        specs = (PartitionSpec("core"),) * (n_params + len(out_avals))
        fn = jax.jit(
            shard_map(_body, mesh=mesh, in_specs=specs,
                      out_specs=(PartitionSpec("core"),) * len(out_names),
                      check_rep=False),
            donate_argnums=donate, keep_unused=True)
        _EXEC_CACHE[key] = (fn, in_names, out_names, zero_outs)

    fn, in_names, out_names, zero_outs = _EXEC_CACHE[key]
    concat_in = [np.concatenate([np.asarray(m[nm]) for m in in_maps], axis=0)
                 for nm in in_names]
    concat_zeros = [np.zeros((n_cores * z.shape[0], *z.shape[1:]), z.dtype)
                    for z in zero_outs]
    out_arrs = fn(*concat_in, *concat_zeros)
    return {nm: np.asarray(a) for nm, a in zip(out_names, out_arrs)}


def kernel(x_in, edge_index, batch, W1, b1, W2, b2):
    n_cores = 8
    pr = preprocess(x_in, edge_index, batch, W1, b1, W2, b2, n_cores)
    nc = build(pr)
    out = exec_spmd(nc, make_in_maps(pr))
    return np.ascontiguousarray(
        out["pool"].reshape(pr.B, pr.K, pr.D).astype(np.float32))


# revision 19
# speedup vs baseline: 40.6395x; 1.0360x over previous
"""DiffPool-like GNN (two GCN convs + softmax clustering + weighted pooling)
as ONE fused Bass/Tile SPMD launch on 8 Trainium2 NeuronCores.

Distribution (matches the sharding hint):
  * nodes partitioned into 8 contiguous shards; each core owns the edges whose
    dst falls in its shard (host buckets edges by 128-node dst window);
  * W1/W2 replicated (const bundle);
  * each core computes h = (D^-1/2 x) @ W rows for ITS OWN shard only, then an
    on-device AllGather assembles the full [Npad, 64] f32 feature table in
    natural node order (the "halo exchange" - here a full gather since edges
    are random);
  * per-edge messages fetched with the MoE dma_gather primitive (256B rows,
    int16 indices -> the table is addressed in 4 parts of Npad/4 rows; padding
    slots point at row 0 and are masked by the one-hot matmul);
  * segment-sum on the tensor engine: per 128-slot chunk, a one-hot matrix
    M[p, r] = (dstloc[p] == r) is built on the vector engine and
    agg += M.T @ msgs accumulates in PSUM across the window's chunks;
  * conv output x1 = dinv*(agg + g_self) + b stays resident in SBUF; the
    conv2 table rows xs1 @ W2 are produced per-window (transpose via an
    identity matmul) and AllGathered the same way;
  * pooling without any gather: per window, onehotB[n, g] = (batch[n] == g)
    over all B=128 graphs and an outer product S[n,k]*x1[n,j] feed
    pooled[g, k*64+j] += onehotB.T @ outer, accumulated in PSUM across all
    windows; a ReduceScatter leaves each core with 16 graph rows ("all-reduce
    the per-(graph,cluster) pooled partial sums");
  * host work: reshape the concatenated ReduceScatter output.

The walrus build in this container encodes at most ONE sync wait per
instruction; _split_waits() rewrites the scheduled BIR, moving excess waits
onto injected single-wait NoOps.
"""

import os
import sys
import numpy as np

sys.path.insert(0, "/opt/trn_rl_repo")

import ml_dtypes  # noqa: E402
import concourse.bacc as bacc  # noqa: E402
import concourse.mybir as mybir  # noqa: E402
import concourse.tile as tile  # noqa: E402
from concourse.tile_rust import add_dep_helper  # noqa: E402

P = 128
BF16 = mybir.dt.bfloat16
F32 = mybir.dt.float32
I16 = mybir.dt.int16
I32 = mybir.dt.int32
NP_BF16 = ml_dtypes.bfloat16

AluOp = mybir.AluOpType
ActFn = mybir.ActivationFunctionType

_DT_MAP = {
    np.dtype(np.float32): F32,
    np.dtype(np.int16): I16,
    np.dtype(NP_BF16): BF16,
}

PARTS = 4
GCAP = 8  # chunks per gather instruction (= 64 descs per engine, HW max)


class ConstBundle:
    """Packs [128, n] arrays of mixed dtypes into one [128, W] int32 array."""

    def __init__(self):
        self.fields = {}
        self.nbytes = 0

    def add(self, name, dtype, n):
        dt = np.dtype(dtype)
        b = dt.itemsize * n
        b4 = (b + 3) & ~3
        self.fields[name] = (self.nbytes, dt, n)
        self.nbytes += b4

    def pack(self, arrays):
        w = self.nbytes // 4
        out = np.zeros((P, w), np.int32)
        ob = out.view(np.uint8)
        for name, (off, dt, n) in self.fields.items():
            a = np.ascontiguousarray(arrays[name])
            assert a.dtype == dt and a.shape == (P, n), (name, a.dtype, a.shape)
            ob[:, off:off + dt.itemsize * n] = a.view(np.uint8)
        return out

    def view(self, cb_sb, name):
        off, dt, n = self.fields[name]
        b4 = (dt.itemsize * n + 3) & ~3
        v = cb_sb[:, off // 4:(off + b4) // 4].bitcast(_DT_MAP[dt])
        return v[:, :n]


def _split_waits(nc, budget=1):
    """Move excess sync waits onto injected single-wait same-engine NoOps.
    The walrus in this container encodes at most one wait per instruction."""
    for fn in nc.m.functions:
        for blk in fn.blocks:
            out = []
            for ins in blk.instructions:
                si = ins.sync_info
                if (si is not None and si.on_wait
                        and len(si.on_wait) > budget
                        and ins.opcode not in ("EventSemaphore",)):
                    waits = list(si.on_wait)
                    excess, keep = waits[:-budget], waits[-budget:]
                    for i, wv in enumerate(excess):
                        nop = mybir.InstNoOp(
                            name=f"{ins.name}-sw{i}", engine=ins.engine,
                            bass_nofuse=True,
                            sync_info=mybir.SyncInfo(on_wait=[wv], on_update=[]))
                        out.append(nop)
                    si.on_wait = keep
                out.append(ins)
            blk.instructions[:] = out


def _wrap16_base(flat):
    """dma_gather index layout base: [16, n/16] int16; index j sits at
    [j%16, j//16]. The device replicates it to all 8 groups (128 rows)."""
    n = flat.shape[0]
    assert n % 16 == 0
    return np.ascontiguousarray(flat.reshape(n // 16, 16).T.astype(np.int16))


def _iota_full():
    return np.tile(np.arange(P, dtype=NP_BF16)[None, :], (P, 1))


# =========================================================================
# host-side preprocessing
# =========================================================================

class Meta:
    pass


def preprocess(x_in, edge_index, batch, W1, b1, W2, b2, n_cores=8):
    pr = Meta()
    N, IN = x_in.shape
    D = W1.shape[1]
    K = W2.shape[1]
    assert IN == P

    src = np.ascontiguousarray(edge_index[0]).astype(np.int64)
    dst = np.ascontiguousarray(edge_index[1]).astype(np.int64)
    batch = np.asarray(batch).astype(np.int64)

    WPC = int(np.ceil(N / n_cores / P))
    NS = WPC * P
    Npad = NS * n_cores
    assert Npad % PARTS == 0
    PS = Npad // PARTS
    assert PS < 2 ** 15

    deg = np.bincount(dst, minlength=N).astype(np.float64)
    dinv_pad = np.ones(Npad, np.float32)
    dinv_pad[:N] = (1.0 / np.sqrt(deg + 1.0)).astype(np.float32)

    pr.__dict__.update(dict(
        N=N, B=P, IN=IN, D=D, K=K, n_cores=n_cores, WPC=WPC, NS=NS,
        Npad=Npad, PS=PS,
        W1=W1.astype(np.float32), b1=b1.astype(np.float32),
        W2=W2.astype(np.float32), b2=b2.astype(np.float32),
    ))

    # ---- per-core xs^T shard (xs = x * dinv), bf16 [IN, NS]
    xs = (x_in * dinv_pad[:N, None]).astype(NP_BF16)
    pr.xT = []
    for c in range(n_cores):
        lo, hi = c * NS, min((c + 1) * NS, N)
        blk = np.zeros((IN, NS), NP_BF16)
        blk[:, :hi - lo] = xs[lo:hi].T
        pr.xT.append(blk)

    pr.dinvT = [np.ascontiguousarray(
        dinv_pad[c * NS:(c + 1) * NS].reshape(WPC, P).T)
        for c in range(n_cores)]

    # batch (global graph id 0..127) per shard slot, -1 for pad rows
    bloc_pad = np.full(Npad, -1.0, np.float32)
    bloc_pad[:N] = batch.astype(np.float32)
    pr.batchlocT = [np.ascontiguousarray(
        bloc_pad[c * NS:(c + 1) * NS].reshape(WPC, P).T).astype(NP_BF16)
        for c in range(n_cores)]

    # ---- global edge slotting by (core, dst window, table part)
    core = dst // NS
    wloc = (dst - core * NS) // P
    part = src // PS
    key = ((core * WPC + wloc) * PARTS + part)
    order = np.argsort(key, kind="stable")
    key_o = key[order]
    cnt = np.bincount(key, minlength=n_cores * WPC * PARTS)
    C4 = max(1, int(np.ceil(cnt.max() / P)))
    SL = C4 * P
    NCHUNK = WPC * PARTS * C4
    pr.C4, pr.NCHUNK = C4, NCHUNK

    starts = np.zeros(n_cores * WPC * PARTS + 1, np.int64)
    np.cumsum(cnt, out=starts[1:])
    slots = np.zeros((n_cores * WPC * PARTS, SL), np.int64)
    dloc = np.full((n_cores * WPC * PARTS, SL), -1.0, np.float32)
    pos = np.arange(len(key_o)) - starts[key_o]
    slots[key_o, pos] = src[order] % PS
    dloc[key_o, pos] = (dst[order] % P).astype(np.float32)

    pr.idx16, pr.dstlocT = [], []
    for c in range(n_cores):
        s = slots[c * WPC * PARTS:(c + 1) * WPC * PARTS]
        d = dloc[c * WPC * PARTS:(c + 1) * WPC * PARTS]
        pr.idx16.append(_wrap16_base(s.reshape(-1)))
        pr.dstlocT.append(np.ascontiguousarray(
            d.reshape(WPC * PARTS, C4, P)
             .transpose(2, 0, 1).reshape(P, NCHUNK)).astype(NP_BF16))

    # ---- const bundle (layout shared across cores)
    cb = ConstBundle()
    cb.add("dinvT", np.float32, WPC)
    cb.add("bt1", np.float32, D)
    cb.add("bt2", np.float32, K)
    cb.add("w1", NP_BF16, D)
    cb.add("w2", NP_BF16, K)
    cb.add("iota", NP_BF16, P)
    cb.add("piota", NP_BF16, 2)
    cb.add("bloc", NP_BF16, WPC)
    cb.add("dstloc", NP_BF16, NCHUNK)
    pr.cb = cb
    return pr


def make_in_maps(pr):
    D, K = pr.D, pr.K
    w2pad = np.zeros((P, K), NP_BF16)
    w2pad[:D] = pr.W2.astype(NP_BF16)
    piota = np.zeros((P, 2), NP_BF16)
    piota[:, 0] = np.arange(P, dtype=NP_BF16)
    maps = []
    for c in range(pr.n_cores):
        cb = pr.cb.pack(dict(
            dinvT=pr.dinvT[c],
            bt1=np.tile(pr.b1[None, :], (P, 1)).astype(np.float32),
            bt2=np.tile(pr.b2[None, :], (P, 1)).astype(np.float32),
            w1=pr.W1.astype(NP_BF16), w2=w2pad,
            iota=_iota_full(), piota=piota,
            bloc=pr.batchlocT[c], dstloc=pr.dstlocT[c]))
        maps.append(dict(xT=pr.xT[c], idx=pr.idx16[c], cb=cb))
    return maps


# =========================================================================
# Bass program builder (single fused launch)
# =========================================================================

def _edge_phase(nc, pools, pr, gtab, idx_sb, dstloc_sb, iota_sb,
                gather_dep, FPAD, FUSE, finish):
    """Per dst-window: PARTS dma_gathers + M-matmul segment sum."""
    WPC, C4, PS = pr.WPC, pr.C4, pr.PS
    msp, mqp, mtp, pp = pools["msgs"], pools["msq"], pools["mt"], pools["ps"]
    first = [True]
    for w in range(WPC):
        mt = mtp.tile([P, PARTS * C4 * P], BF16)
        nc.vector.tensor_tensor(
            out=mt[:].rearrange("p (k r) -> p k r", r=P),
            in0=dstloc_sb[:, w * PARTS * C4:(w + 1) * PARTS * C4]
                .unsqueeze(2).to_broadcast([P, PARTS * C4, P]),
            in1=iota_sb[:].unsqueeze(1).to_broadcast([P, PARTS * C4, P]),
            op=AluOp.is_equal)
        pst = pp.tile([P, 512], F32, name="pst", tag="seg")
        ps = pst[:, :FUSE]
        for q in range(PARTS):
            for k0 in range(0, C4, GCAP):
                nk = min(GCAP, C4 - k0)
                cc0 = (w * PARTS + q) * C4 + k0
                msgs = msp.tile([P, GCAP * FPAD], F32)
                g = nc.gpsimd.dma_gather(
                    msgs[:, :nk * FPAD].rearrange("p (c e) -> p c e", e=FPAD),
                    gtab[q * PS:(q + 1) * PS, :],
                    idx_sb[:, cc0 * 8:(cc0 + nk) * 8],
                    nk * P, nk * P, FPAD)
                if first[0]:
                    first[0] = False
                    add_dep_helper(g.ins, gather_dep.ins, sync=True,
                                   reason="gather after table allgather")
                msq = mqp.tile([P, GCAP * FUSE], BF16)
                if FUSE == FPAD:
                    nc.scalar.copy(out=msq[:, :nk * FUSE],
                                   in_=msgs[:, :nk * FPAD])
                else:
                    nc.scalar.copy(
                        out=msq[:, :nk * FUSE]
                            .rearrange("p (c e) -> p c e", e=FUSE),
                        in_=msgs[:, :nk * FPAD]
                            .rearrange("p (c e) -> p c e", e=FPAD)[:, :, :FUSE])
                for k in range(nk):
                    nc.tensor.matmul(
                        ps[:],
                        lhsT=mt[:, (q * C4 + k0 + k) * P:
                                   (q * C4 + k0 + k + 1) * P],
                        rhs=msq[:, k * FUSE:(k + 1) * FUSE],
                        start=(q == 0 and k0 == 0 and k == 0),
                        stop=(q == PARTS - 1 and k0 + k == C4 - 1))
        finish(w, ps)


def build(pr, split=True, reps=1):
    from contextlib import ExitStack
    IN, D, K, WPC, Npad, NS, PS = (pr.IN, pr.D, pr.K, pr.WPC, pr.Npad,
                                   pr.NS, pr.PS)
    NCHUNK, C4 = pr.NCHUNK, pr.C4
    CBW = pr.cb.nbytes // 4
    IDXW = NCHUNK * 8
    KD = K * D
    TB = 7
    assert WPC % TB == 0

    nc = bacc.Bacc("TRN2")
    xT_d = nc.declare_dram_parameter("xT", [IN, NS], BF16, isOutput=False)
    idx_d = nc.declare_dram_parameter("idx", [16, IDXW], I16, isOutput=False)
    cb_d = nc.declare_dram_parameter("cb", [P, CBW], I32, isOutput=False)
    pool_d = nc.declare_dram_parameter("pool", [P // 8, KD], F32,
                                       isOutput=True)

    gtab1 = nc.dram_tensor("gtab1", [Npad, D], F32)
    gtab2 = nc.dram_tensor("gtab2", [Npad, D], F32)
    ag1_in = nc.dram_tensor("ag1in", [NS, D], F32)
    ag2_in = nc.dram_tensor("ag2in", [NS, D], F32)
    rs_in = nc.dram_tensor("rsin", [P, KD], F32)
    rs_out = nc.dram_tensor("rsout", [P // 8, KD], F32)

    with tile.TileContext(nc) as tc, ExitStack() as es:
        pools = {}
        for nm, bufs, space in [
                ("const", 1, None), ("msgs", 4, None), ("msq", 4, None),
                ("mt", 2, None), ("xw", 4, None), ("hw", 3, None),
                ("ps", 2, "PSUM"), ("psb", 2, "PSUM")]:
            kw = dict(name=nm, bufs=bufs)
            if space:
                kw["space"] = space
            pools[nm] = es.enter_context(tc.tile_pool(**kw))
        cp = pools["const"]

        cb_sb = cp.tile([P, CBW], I32, name="cb_sb", tag="cb_sb")
        nc.sync.dma_start(out=cb_sb[:], in_=cb_d[:])
        nc.vector.tensor_copy(out=cb_sb[:], in_=cb_sb[:])
        V = lambda name: pr.cb.view(cb_sb, name)
        dinvT_sb, iota_sb = V("dinvT"), V("iota")

        idx_sb = cp.tile([P, IDXW], I16, name="idx_sb", tag="idx_sb")
        for g in range(8):
            nc.sync.dma_start(out=idx_sb[g * 16:(g + 1) * 16, :], in_=idx_d[:])

        g1_shard = cp.tile([P, WPC * D], F32)
        g2_shard = cp.tile([P, WPC * K], F32)
        x1_sb = cp.tile([P, WPC * D], BF16)
        ident = cp.tile([P, P], BF16, name="ident", tag="ident")
        nc.vector.tensor_tensor(
            out=ident[:],
            in0=V("piota")[:, :1].to_broadcast([P, P]),
            in1=iota_sb[:], op=AluOp.is_equal)

        bt1_sb, w2_sb = V("bt1"), V("w2")
        bt2_sb, bloc_sb = V("bt2"), V("bloc")
        xwp, hwp = pools["xw"], pools["hw"]
        plq_pool = es.enter_context(tc.tile_pool(name="plq", bufs=1,
                                                 space="PSUM"))
        NQ = KD // 512
        state = {}

        def phase1(r):
            # ---- own-shard h1 rows -> ag1_in; AllGather -> gtab1
            with tc.tile_pool(name=f"xt{r}", bufs=1) as xtp:
                xt = xtp.tile([IN, NS], BF16, name="xt", tag="xt")
                nc.sync.dma_start(out=xt[:], in_=xT_d[:])
                w1_sb = V("w1")
                writes = []
                for gi in range(WPC // TB):
                    ps8 = pools["ps"].tile([P, 512], F32, name="ps8",
                                           tag="seg")
                    for j in range(TB):
                        t = gi * TB + j
                        nc.tensor.matmul(ps8[:, j * D:(j + 1) * D],
                                         lhsT=xt[:, t * P:(t + 1) * P],
                                         rhs=w1_sb[:], start=True, stop=True)
                    nc.vector.tensor_copy(
                        out=g1_shard[:, gi * TB * D:(gi + 1) * TB * D],
                        in_=ps8[:, :TB * D])
                    w = nc.sync.dma_start(
                        out=ag1_in[gi * TB * P:(gi + 1) * TB * P, :]
                            .rearrange("(t p) d -> p t d", p=P),
                        in_=g1_shard[:, gi * TB * D:(gi + 1) * TB * D]
                            .rearrange("p (t d) -> p t d", d=D))
                    writes.append(w)
            ag1 = nc.gpsimd.collective_compute(
                "AllGather", AluOp.bypass,
                replica_groups=[list(range(pr.n_cores))],
                ins=[ag1_in[:, :]], outs=[gtab1[:, :]])
            for w in writes:
                add_dep_helper(ag1.ins, w.ins, sync=True, reason="ag1 after h1")
            return ag1

        def finish1(w, ps):
            h2_writes = state["h2_writes"]
            t1 = xwp.tile([P, D], F32, tag="t1")
            nc.vector.tensor_tensor(out=t1[:], in0=ps[:],
                                    in1=g1_shard[:, w * D:(w + 1) * D],
                                    op=AluOp.add)
            xf = xwp.tile([P, D], F32, tag="xf")
            nc.vector.tensor_scalar(
                out=xf[:], in0=t1[:], scalar1=dinvT_sb[:, w:w + 1],
                scalar2=None, op0=AluOp.mult)
            nc.vector.tensor_tensor(out=x1_sb[:, w * D:(w + 1) * D],
                                    in0=xf[:], in1=bt1_sb[:], op=AluOp.add)
            xsq = xwp.tile([P, D], BF16, tag="xsq")
            nc.vector.tensor_scalar(
                out=xsq[:], in0=x1_sb[:, w * D:(w + 1) * D],
                scalar1=dinvT_sb[:, w:w + 1], scalar2=None, op0=AluOp.mult)
            # transpose xs1 via identity matmul, then h2 = xs1 @ W2
            pstt = pools["psb"].tile([P, 512], F32, name="pstt", tag="psb")
            pst = pstt[:D, :P]
            nc.tensor.matmul(pst, lhsT=xsq[:], rhs=ident[:],
                             start=True, stop=True)
            xst = xwp.tile([D, P], BF16, tag="xst")
            nc.scalar.copy(out=xst[:], in_=pst)
            ph2t = pools["psb"].tile([P, 512], F32, name="ph2t", tag="psb")
            ph2 = ph2t[:, :K]
            nc.tensor.matmul(ph2, lhsT=xst[:], rhs=w2_sb[:D, :],
                             start=True, stop=True)
            nc.vector.tensor_copy(out=g2_shard[:, w * K:(w + 1) * K],
                                  in_=ph2)
            h8 = hwp.tile([P, D], F32)
            nc.scalar.copy(out=h8[:, :K], in_=ph2)
            nc.scalar.activation(out=h8[:, K:], in_=ph2,
                                 func=ActFn.Copy, scale=0.0)
            hw_ = nc.sync.dma_start(out=ag2_in[w * P:(w + 1) * P, :],
                                    in_=h8[:])
            h2_writes.append(hw_)

        def phase2(r, ag1):
            # ---- conv1 edge phase
            state["h2_writes"] = []
            _edge_phase(nc, pools, pr, gtab1, idx_sb, V("dstloc"), iota_sb,
                        ag1, D, D, finish1)
            ag2 = nc.gpsimd.collective_compute(
                "AllGather", AluOp.bypass,
                replica_groups=[list(range(pr.n_cores))],
                ins=[ag2_in[:, :]], outs=[gtab2[:, :]])
            for w in state["h2_writes"]:
                add_dep_helper(ag2.ins, w.ins, sync=True, reason="ag2 after h2")
            return ag2

        def finish2(w, ps):
            plq = state["plq"]
            t1 = xwp.tile([P, K], F32, tag="t1")
            nc.vector.tensor_tensor(out=t1[:], in0=ps[:],
                                    in1=g2_shard[:, w * K:(w + 1) * K],
                                    op=AluOp.add)
            sl = xwp.tile([P, K], F32, tag="xf")
            nc.vector.tensor_scalar(
                out=sl[:], in0=t1[:], scalar1=dinvT_sb[:, w:w + 1],
                scalar2=None, op0=AluOp.mult)
            sl2 = xwp.tile([P, K], F32, tag="sl2")
            nc.vector.tensor_tensor(out=sl2[:], in0=sl[:], in1=bt2_sb[:],
                                    op=AluOp.add)
            ex = xwp.tile([P, K], F32, tag="ex")
            nc.scalar.activation(out=ex[:], in_=sl2[:], func=ActFn.Exp)
            sm = xwp.tile([P, 1], F32, tag="sm")
            nc.vector.tensor_reduce(out=sm[:], in_=ex[:],
                                    axis=mybir.AxisListType.X, op=AluOp.add)
            rc = xwp.tile([P, 1], F32, tag="rc")
            nc.vector.reciprocal(out=rc[:], in_=sm[:])
            sq = xwp.tile([P, K], BF16, tag="sq")
            nc.vector.tensor_scalar(
                out=sq[:], in0=ex[:], scalar1=rc[:, :1], scalar2=None,
                op0=AluOp.mult)
            ob = xwp.tile([P, P], BF16, tag="ob")
            nc.vector.tensor_tensor(
                out=ob[:], in0=bloc_sb[:, w:w + 1].to_broadcast([P, P]),
                in1=iota_sb[:], op=AluOp.is_equal)
            outer = hwp.tile([P, KD], BF16)
            nc.vector.tensor_tensor(
                out=outer[:].rearrange("p (k j) -> p k j", j=D),
                in0=sq[:].unsqueeze(2).to_broadcast([P, K, D]),
                in1=x1_sb[:, w * D:(w + 1) * D]
                    .unsqueeze(1).to_broadcast([P, K, D]),
                op=AluOp.mult)
            for i in range(NQ):
                nc.tensor.matmul(plq[i][:], lhsT=ob[:],
                                 rhs=outer[:, i * 512:(i + 1) * 512],
                                 start=(w == 0), stop=(w == WPC - 1))

        def phase3(r, ag2):
            # ---- conv2 edge phase + pooling accumulation + ReduceScatter
            state["plq"] = [
                plq_pool.tile([P, 512], F32, name=f"plq{i}", tag=f"plq{i}")
                for i in range(NQ)]
            _edge_phase(nc, pools, pr, gtab2, idx_sb, V("dstloc"), iota_sb,
                        ag2, D, K, finish2)
            pool_sb = cp.tile([P, KD], F32, name=f"pool_sb{r}",
                              tag=f"pool_sb{r}")
            for i in range(NQ):
                nc.vector.tensor_copy(out=pool_sb[:, i * 512:(i + 1) * 512],
                                      in_=state["plq"][i][:])
            rw = nc.sync.dma_start(out=rs_in[:, :], in_=pool_sb[:])
            rs = nc.gpsimd.collective_compute(
                "ReduceScatter", AluOp.add,
                replica_groups=[list(range(pr.n_cores))],
                ins=[rs_in[:, :]], outs=[rs_out[:, :]])
            add_dep_helper(rs.ins, rw.ins, sync=True, reason="rs after pool")
            return rs

        rs = None
        for r in range(reps):
            ag1 = phase1(r)
            if rs is not None:  # serialize reps (slope microbenchmark)
                add_dep_helper(ag1.ins, rs.ins, sync=True, reason="rep chain")
            ag2 = phase2(r, ag1)
            rs = phase3(r, ag2)

        out_sb = cp.tile([P // 8, KD], F32, name="out_sb", tag="out_sb")
        rd = nc.sync.dma_start(out=out_sb[:], in_=rs_out[:, :])
        add_dep_helper(rd.ins, rs.ins, sync=True, reason="read after rs")
        nc.sync.dma_start(out=pool_d[:], in_=out_sb[:])
    nc.compile()
    if split:
        _split_waits(nc)
    return nc


# =========================================================================
# runner + glue
# =========================================================================

_EXEC_CACHE = {}


def exec_spmd(nc, in_maps):
    """Execute a prebuilt Bass module on len(in_maps) cores via PJRT.

    Mirrors concourse.bass2jax.run_bass_via_pjrt, but (a) caches the jitted
    callable per-module so repeated runs don't re-trace/re-compile XLA, and
    (b) fetches each output as ONE global [n_cores*rows, cols] array (one
    device round-trip) instead of per-core sliced fetches.
    Returns {name: global np.ndarray} with per-core rows concatenated.
    """
    import jax
    from jax.sharding import Mesh, PartitionSpec
    from jax.experimental.shard_map import shard_map
    from concourse import bass2jax, mybir as _mybir
    from concourse.bass2jax import (_bass_exec_p, install_neuronx_cc_hook,
                                    partition_id_tensor)

    n_cores = len(in_maps)
    key = id(nc)
    if key not in _EXEC_CACHE:
        install_neuronx_cc_hook()
        assert nc.dbg_addr is None or not nc.dbg_callbacks
        partition_name = (nc.partition_id_tensor.name
                          if nc.partition_id_tensor else None)
        in_names, out_names, out_avals, zero_outs = [], [], [], []
        for alloc in nc.m.functions[0].allocations:
            if not isinstance(alloc, _mybir.MemoryLocationSet):
                continue
            name = alloc.memorylocations[0].name
            if alloc.kind == "ExternalInput":
                if name != partition_name:
                    in_names.append(name)
            elif alloc.kind == "ExternalOutput":
                shape = tuple(alloc.tensor_shape)
                dtype = _mybir.dt.np(alloc.dtype)
                out_names.append(name)
                out_avals.append(jax.core.ShapedArray(shape, dtype))
                zero_outs.append(np.zeros(shape, dtype))
        n_params = len(in_names)
        all_in = list(in_names) + list(out_names)
        if partition_name is not None:
            all_in.append(partition_name)
        donate = tuple(range(n_params, n_params + len(out_avals)))

        def _body(*args):
            operands = list(args)
            if partition_name is not None:
                operands.append(partition_id_tensor())
            return tuple(_bass_exec_p.bind(
                *operands, out_avals=tuple(out_avals), in_names=tuple(all_in),
                out_names=tuple(out_names), lowering_input_output_aliases=(),
                sim_require_finite=True, sim_require_nnan=True, nc=nc))

        mesh = Mesh(np.asarray(jax.devices()[:n_cores]), ("core",))
        specs = (PartitionSpec("core"),) * (n_params + len(out_avals))
        fn = jax.jit(
            shard_map(_body, mesh=mesh, in_specs=specs,
                      out_specs=(PartitionSpec("core"),) * len(out_names),
                      check_rep=False),
            donate_argnums=donate, keep_unused=True)
        _EXEC_CACHE[key] = (fn, in_names, out_names, zero_outs)

    fn, in_names, out_names, zero_outs = _EXEC_CACHE[key]
    concat_in = [np.concatenate([np.asarray(m[nm]) for m in in_maps], axis=0)
                 for nm in in_names]
    concat_zeros = [np.zeros((n_cores * z.shape[0], *z.shape[1:]), z.dtype)
                    for z in zero_outs]
    out_arrs = fn(*concat_in, *concat_zeros)
    return {nm: np.asarray(a) for nm, a in zip(out_names, out_arrs)}


def kernel(x_in, edge_index, batch, W1, b1, W2, b2):
    n_cores = 8
    pr = preprocess(x_in, edge_index, batch, W1, b1, W2, b2, n_cores)
    nc = build(pr)
    out = exec_spmd(nc, make_in_maps(pr))
    return np.ascontiguousarray(
        out["pool"].reshape(pr.B, pr.K, pr.D).astype(np.float32))


# revision 29
# speedup vs baseline: 7909.6489x; 194.6297x over previous
"""DiffPool-like GNN (two GCN convs + softmax clustering + weighted pooling)
as ONE fused Bass/Tile SPMD launch on 8 Trainium2 NeuronCores.

Distribution (matches the sharding hint):
  * nodes partitioned into 8 contiguous shards; each core owns the edges whose
    dst falls in its shard (host buckets edges by 128-node dst window);
  * W1/W2 replicated (const bundle);
  * each core computes h = (D^-1/2 x) @ W rows for ITS OWN shard only, then an
    on-device AllGather assembles the full [Npad, 64] f32 feature table in
    natural node order (the "halo exchange" - here a full gather since edges
    are random);
  * per-edge messages fetched with the MoE dma_gather primitive (256B rows,
    int16 indices -> the table is addressed in 4 parts of Npad/4 rows; padding
    slots point at row 0 and are masked by the one-hot matmul);
  * segment-sum on the tensor engine: per 128-slot chunk, a one-hot matrix
    M[p, r] = (dstloc[p] == r) is built on the vector engine and
    agg += M.T @ msgs accumulates in PSUM across the window's chunks;
  * conv output x1 = dinv*(agg + g_self) + b stays resident in SBUF; the
    conv2 table rows xs1 @ W2 are produced per-window (transpose via an
    identity matmul) and AllGathered the same way;
  * pooling without any gather: per window, onehotB[n, g] = (batch[n] == g)
    over all B=128 graphs and an outer product S[n,k]*x1[n,j] feed
    pooled[g, k*64+j] += onehotB.T @ outer, accumulated in PSUM across all
    windows; a ReduceScatter leaves each core with 16 graph rows ("all-reduce
    the per-(graph,cluster) pooled partial sums");
  * host work: reshape the concatenated ReduceScatter output.

The walrus build in this container encodes at most ONE sync wait per
instruction; _split_waits() rewrites the scheduled BIR, moving excess waits
onto injected single-wait NoOps.
"""

import os
import sys
import numpy as np

sys.path.insert(0, "/opt/trn_rl_repo")

import ml_dtypes  # noqa: E402
import concourse.bacc as bacc  # noqa: E402
import concourse.mybir as mybir  # noqa: E402
import concourse.tile as tile  # noqa: E402
from concourse.tile_rust import add_dep_helper  # noqa: E402

P = 128
BF16 = mybir.dt.bfloat16
F32 = mybir.dt.float32
I16 = mybir.dt.int16
I32 = mybir.dt.int32
NP_BF16 = ml_dtypes.bfloat16

AluOp = mybir.AluOpType
ActFn = mybir.ActivationFunctionType

_DT_MAP = {
    np.dtype(np.float32): F32,
    np.dtype(np.int16): I16,
    np.dtype(np.int8): mybir.dt.int8,
    np.dtype(NP_BF16): BF16,
}

PARTS = 4
GCAP = 8  # chunks per gather instruction (= 64 descs per engine, HW max)


class ConstBundle:
    """Packs [128, n] arrays of mixed dtypes into one [128, W] int32 array."""

    def __init__(self):
        self.fields = {}
        self.nbytes = 0

    def add(self, name, dtype, n):
        dt = np.dtype(dtype)
        b = dt.itemsize * n
        b4 = (b + 3) & ~3
        self.fields[name] = (self.nbytes, dt, n)
        self.nbytes += b4

    def pack(self, arrays):
        w = self.nbytes // 4
        out = np.zeros((P, w), np.int32)
        ob = out.view(np.uint8)
        for name, (off, dt, n) in self.fields.items():
            a = np.ascontiguousarray(arrays[name])
            assert a.dtype == dt and a.shape == (P, n), (name, a.dtype, a.shape)
            ob[:, off:off + dt.itemsize * n] = a.view(np.uint8)
        return out

    def view(self, cb_sb, name):
        off, dt, n = self.fields[name]
        b4 = (dt.itemsize * n + 3) & ~3
        v = cb_sb[:, off // 4:(off + b4) // 4].bitcast(_DT_MAP[dt])
        return v[:, :n]


def _split_waits(nc, budget=1):
    """Move excess sync waits onto injected single-wait same-engine NoOps.
    The walrus in this container encodes at most one wait per instruction."""
    for fn in nc.m.functions:
        for blk in fn.blocks:
            out = []
            for ins in blk.instructions:
                si = ins.sync_info
                if (si is not None and si.on_wait
                        and len(si.on_wait) > budget
                        and ins.opcode not in ("EventSemaphore",)):
                    waits = list(si.on_wait)
                    excess, keep = waits[:-budget], waits[-budget:]
                    for i, wv in enumerate(excess):
                        nop = mybir.InstNoOp(
                            name=f"{ins.name}-sw{i}", engine=ins.engine,
                            bass_nofuse=True,
                            sync_info=mybir.SyncInfo(on_wait=[wv], on_update=[]))
                        out.append(nop)
                    si.on_wait = keep
                out.append(ins)
            blk.instructions[:] = out


def _wrap16_base(flat):
    """dma_gather index layout base: [16, n/16] int16; index j sits at
    [j%16, j//16]. The device replicates it to all 8 groups (128 rows)."""
    n = flat.shape[0]
    assert n % 16 == 0
    return np.ascontiguousarray(flat.reshape(n // 16, 16).T.astype(np.int16))


def _iota_full():
    return np.tile(np.arange(P, dtype=NP_BF16)[None, :], (P, 1))


# =========================================================================
# host-side preprocessing
# =========================================================================

class Meta:
    pass


def preprocess(x_in, edge_index, batch, W1, b1, W2, b2, n_cores=8):
    pr = Meta()
    N, IN = x_in.shape
    D = W1.shape[1]
    K = W2.shape[1]
    assert IN == P

    src = np.ascontiguousarray(edge_index[0]).astype(np.int64)
    dst = np.ascontiguousarray(edge_index[1]).astype(np.int64)
    batch = np.asarray(batch).astype(np.int64)

    WPC = int(np.ceil(N / n_cores / P))
    NS = WPC * P
    Npad = NS * n_cores
    assert Npad % PARTS == 0
    PS = Npad // PARTS
    assert PS < 2 ** 15

    deg = np.bincount(dst, minlength=N).astype(np.float64)
    dinv_pad = np.ones(Npad, np.float32)
    dinv_pad[:N] = (1.0 / np.sqrt(deg + 1.0)).astype(np.float32)

    pr.__dict__.update(dict(
        N=N, B=P, IN=IN, D=D, K=K, n_cores=n_cores, WPC=WPC, NS=NS,
        Npad=Npad, PS=PS,
        W1=W1.astype(np.float32), b1=b1.astype(np.float32),
        W2=W2.astype(np.float32), b2=b2.astype(np.float32),
    ))

    # ---- per-core xs^T shard (xs = x * dinv), bf16 [IN, NS]
    xs = (x_in * dinv_pad[:N, None]).astype(NP_BF16)
    pr.xT = []
    for c in range(n_cores):
        lo, hi = c * NS, min((c + 1) * NS, N)
        blk = np.zeros((IN, NS), NP_BF16)
        blk[:, :hi - lo] = xs[lo:hi].T
        pr.xT.append(blk)

    pr.dinvT = [np.ascontiguousarray(
        dinv_pad[c * NS:(c + 1) * NS].reshape(WPC, P).T)
        for c in range(n_cores)]

    # batch (global graph id 0..127) per shard slot, -1 for pad rows
    bloc_pad = np.full(Npad, -1.0, np.float32)
    bloc_pad[:N] = batch.astype(np.float32)
    pr.batchlocT = [np.ascontiguousarray(
        bloc_pad[c * NS:(c + 1) * NS].reshape(WPC, P).T).astype(NP_BF16)
        for c in range(n_cores)]

    # ---- global edge slotting by (core, dst window, table part)
    core = dst // NS
    wloc = (dst - core * NS) // P
    part = src // PS
    key = ((core * WPC + wloc) * PARTS + part)
    order = np.argsort(key, kind="stable")
    key_o = key[order]
    cnt = np.bincount(key, minlength=n_cores * WPC * PARTS)
    C4 = max(1, int(np.ceil(cnt.max() / P)))
    SL = C4 * P
    NCHUNK = WPC * PARTS * C4
    pr.C4, pr.NCHUNK = C4, NCHUNK

    starts = np.zeros(n_cores * WPC * PARTS + 1, np.int64)
    np.cumsum(cnt, out=starts[1:])
    slots = np.zeros((n_cores * WPC * PARTS, SL), np.int64)
    dloc = np.full((n_cores * WPC * PARTS, SL), -1.0, np.float32)
    pos = np.arange(len(key_o)) - starts[key_o]
    slots[key_o, pos] = src[order] % PS
    dloc[key_o, pos] = (dst[order] % P).astype(np.float32)

    pr.idx16, pr.dstlocT = [], []
    for c in range(n_cores):
        s = slots[c * WPC * PARTS:(c + 1) * WPC * PARTS]
        d = dloc[c * WPC * PARTS:(c + 1) * WPC * PARTS]
        pr.idx16.append(_wrap16_base(s.reshape(-1)))
        pr.dstlocT.append(np.ascontiguousarray(
            d.reshape(WPC * PARTS, C4, P)
             .transpose(2, 0, 1).reshape(P, NCHUNK)).astype(np.int8))

    # ---- const bundle (layout shared across cores)
    cb = ConstBundle()
    cb.add("dinvT", np.float32, WPC)
    cb.add("bt1", np.float32, D)
    cb.add("bt2", np.float32, K)
    cb.add("w1", NP_BF16, D)
    cb.add("w2", NP_BF16, K)
    cb.add("iota", NP_BF16, P)
    cb.add("piota", NP_BF16, 2)
    cb.add("bloc", NP_BF16, WPC)
    cb.add("dstloc", np.int8, NCHUNK)
    pr.cb = cb
    return pr


def make_in_maps(pr):
    D, K = pr.D, pr.K
    w2pad = np.zeros((P, K), NP_BF16)
    w2pad[:D] = pr.W2.astype(NP_BF16)
    piota = np.zeros((P, 2), NP_BF16)
    piota[:, 0] = np.arange(P, dtype=NP_BF16)
    maps = []
    for c in range(pr.n_cores):
        cb = pr.cb.pack(dict(
            dinvT=pr.dinvT[c],
            bt1=np.tile(pr.b1[None, :], (P, 1)).astype(np.float32),
            bt2=np.tile(pr.b2[None, :], (P, 1)).astype(np.float32),
            w1=pr.W1.astype(NP_BF16), w2=w2pad,
            iota=_iota_full(), piota=piota,
            bloc=pr.batchlocT[c], dstloc=pr.dstlocT[c]))
        maps.append(dict(xT=pr.xT[c], idx=pr.idx16[c], cb=cb))
    return maps


# =========================================================================
# Bass program builder (single fused launch)
# =========================================================================

def _edge_phase(nc, pools, pr, gtab, idxrep_d, dstloc_sb, iota_sb,
                gather_dep, FPAD, FUSE, finish):
    """Per dst-window: PARTS dma_gathers + M-matmul segment sum."""
    WPC, C4, PS = pr.WPC, pr.C4, pr.PS
    msp, mqp, mtp, pp, ixp = (pools["msgs"], pools["msq"], pools["mt"],
                              pools["ps"], pools["ix"])
    WIX = PARTS * C4 * 8  # idx cols per window
    first = [True]
    for w in range(WPC):
        idxw = ixp.tile([P, WIX], I16)
        nc.sync.dma_start(out=idxw[:],
                          in_=idxrep_d[:, w * WIX:(w + 1) * WIX])
        mt = mtp.tile([P, PARTS * C4 * P], BF16)
        nc.vector.tensor_tensor(
            out=mt[:].rearrange("p (k r) -> p k r", r=P),
            in0=dstloc_sb[:, w * PARTS * C4:(w + 1) * PARTS * C4]
                .unsqueeze(2).to_broadcast([P, PARTS * C4, P]),
            in1=iota_sb[:].unsqueeze(1).to_broadcast([P, PARTS * C4, P]),
            op=AluOp.is_equal)
        pst = pp.tile([P, 512], F32, name="pst", tag="seg")
        ps = pst[:, :FUSE]
        for q in range(PARTS):
            for k0 in range(0, C4, GCAP):
                nk = min(GCAP, C4 - k0)
                lc0 = q * C4 + k0
                msgs = msp.tile([P, GCAP * FPAD], F32)
                g = nc.gpsimd.dma_gather(
                    msgs[:, :nk * FPAD].rearrange("p (c e) -> p c e", e=FPAD),
                    gtab[q * PS:(q + 1) * PS, :],
                    idxw[:, lc0 * 8:(lc0 + nk) * 8],
                    nk * P, nk * P, FPAD)
                if first[0]:
                    first[0] = False
                    add_dep_helper(g.ins, gather_dep.ins, sync=True,
                                   reason="gather after table allgather")
                msq = mqp.tile([P, GCAP * FUSE], BF16)
                if FUSE == FPAD:
                    nc.scalar.copy(out=msq[:, :nk * FUSE],
                                   in_=msgs[:, :nk * FPAD])
                else:
                    nc.scalar.copy(
                        out=msq[:, :nk * FUSE]
                            .rearrange("p (c e) -> p c e", e=FUSE),
                        in_=msgs[:, :nk * FPAD]
                            .rearrange("p (c e) -> p c e", e=FPAD)[:, :, :FUSE])
                for k in range(nk):
                    nc.tensor.matmul(
                        ps[:],
                        lhsT=mt[:, (q * C4 + k0 + k) * P:
                                   (q * C4 + k0 + k + 1) * P],
                        rhs=msq[:, k * FUSE:(k + 1) * FUSE],
                        start=(q == 0 and k0 == 0 and k == 0),
                        stop=(q == PARTS - 1 and k0 + k == C4 - 1))
        finish(w, ps)


def build(pr, split=True, reps=1):
    from contextlib import ExitStack
    IN, D, K, WPC, Npad, NS, PS = (pr.IN, pr.D, pr.K, pr.WPC, pr.Npad,
                                   pr.NS, pr.PS)
    NCHUNK, C4 = pr.NCHUNK, pr.C4
    CBW = pr.cb.nbytes // 4
    IDXW = NCHUNK * 8
    KD = K * D
    TB = 7
    assert WPC % TB == 0

    nc = bacc.Bacc("TRN2")
    xT_d = nc.declare_dram_parameter("xT", [IN, NS], BF16, isOutput=False)
    idx_d = nc.declare_dram_parameter("idx", [16, IDXW], I16, isOutput=False)
    cb_d = nc.declare_dram_parameter("cb", [P, CBW], I32, isOutput=False)
    pool_d = nc.declare_dram_parameter("pool", [P // 8, KD], F32,
                                       isOutput=True)

    gtab1 = nc.dram_tensor("gtab1", [Npad, D], F32)
    gtab2 = nc.dram_tensor("gtab2", [Npad, D], F32)
    idxrep_d = nc.dram_tensor("idxrep", [P, IDXW], I16)
    ag1_in = nc.dram_tensor("ag1in", [NS, D], F32)
    ag2_in = nc.dram_tensor("ag2in", [NS, D], F32)
    rs_in = nc.dram_tensor("rsin", [P, KD], F32)
    rs_out = nc.dram_tensor("rsout", [P // 8, KD], F32)

    with tile.TileContext(nc) as tc, ExitStack() as es:
        pools = {}
        for nm, bufs, space in [
                ("const", 1, None), ("msgs", 6, None), ("msq", 6, None),
                ("mt", 3, None), ("xw", 4, None), ("hw", 3, None),
                ("ix", 3, None),
                ("ps", 2, "PSUM"), ("psb", 2, "PSUM")]:
            kw = dict(name=nm, bufs=bufs)
            if space:
                kw["space"] = space
            pools[nm] = es.enter_context(tc.tile_pool(**kw))
        cp = pools["const"]

        cb_sb = cp.tile([P, CBW], I32, name="cb_sb", tag="cb_sb")
        nc.sync.dma_start(out=cb_sb[:], in_=cb_d[:])
        nc.vector.tensor_copy(out=cb_sb[:], in_=cb_sb[:])
        V = lambda name: pr.cb.view(cb_sb, name)
        dinvT_sb, iota_sb = V("dinvT"), V("iota")

        # replicate the 16-row idx base to all 128 partition rows, in DRAM;
        # per-window slices stream back into small SBUF tiles in _edge_phase
        for g in range(8):
            nc.sync.dma_start(out=idxrep_d[g * 16:(g + 1) * 16, :],
                              in_=idx_d[:])

        dstloc_sb = cp.tile([P, NCHUNK], BF16, name="dstloc_sb",
                            tag="dstloc_sb")
        nc.vector.tensor_copy(out=dstloc_sb[:], in_=V("dstloc"))

        g1_shard = cp.tile([P, WPC * D], F32)
        g2_shard = cp.tile([P, WPC * K], F32)
        x1_sb = cp.tile([P, WPC * D], BF16)
        ident = cp.tile([P, P], BF16, name="ident", tag="ident")
        nc.vector.tensor_tensor(
            out=ident[:],
            in0=V("piota")[:, :1].to_broadcast([P, P]),
            in1=iota_sb[:], op=AluOp.is_equal)

        bt1_sb, w2_sb = V("bt1"), V("w2")
        bt2_sb, bloc_sb = V("bt2"), V("bloc")
        xwp, hwp = pools["xw"], pools["hw"]
        plq_pool = es.enter_context(tc.tile_pool(name="plq", bufs=1,
                                                 space="PSUM"))
        NQ = KD // 512
        state = {}

        def phase1(r):
            # ---- own-shard h1 rows -> ag1_in; AllGather -> gtab1
            with tc.tile_pool(name=f"xt{r}", bufs=1) as xtp:
                xt = xtp.tile([IN, NS], BF16, name="xt", tag="xt")
                nc.sync.dma_start(out=xt[:], in_=xT_d[:])
                w1_sb = V("w1")
                writes = []
                for gi in range(WPC // TB):
                    ps8 = pools["ps"].tile([P, 512], F32, name="ps8",
                                           tag="seg")
                    for j in range(TB):
                        t = gi * TB + j
                        nc.tensor.matmul(ps8[:, j * D:(j + 1) * D],
                                         lhsT=xt[:, t * P:(t + 1) * P],
                                         rhs=w1_sb[:], start=True, stop=True)
                    nc.vector.tensor_copy(
                        out=g1_shard[:, gi * TB * D:(gi + 1) * TB * D],
                        in_=ps8[:, :TB * D])
                    w = nc.sync.dma_start(
                        out=ag1_in[gi * TB * P:(gi + 1) * TB * P, :]
                            .rearrange("(t p) d -> p t d", p=P),
                        in_=g1_shard[:, gi * TB * D:(gi + 1) * TB * D]
                            .rearrange("p (t d) -> p t d", d=D))
                    writes.append(w)
            ag1 = nc.gpsimd.collective_compute(
                "AllGather", AluOp.bypass,
                replica_groups=[list(range(pr.n_cores))],
                ins=[ag1_in[:, :]], outs=[gtab1[:, :]])
            for w in writes:
                add_dep_helper(ag1.ins, w.ins, sync=True, reason="ag1 after h1")
            return ag1

        def finish1(w, ps):
            h2_writes = state["h2_writes"]
            t1 = xwp.tile([P, D], F32, tag="t1")
            nc.vector.tensor_tensor(out=t1[:], in0=ps[:],
                                    in1=g1_shard[:, w * D:(w + 1) * D],
                                    op=AluOp.add)
            xf = xwp.tile([P, D], F32, tag="xf")
            nc.vector.tensor_scalar(
                out=xf[:], in0=t1[:], scalar1=dinvT_sb[:, w:w + 1],
                scalar2=None, op0=AluOp.mult)
            nc.vector.tensor_tensor(out=x1_sb[:, w * D:(w + 1) * D],
                                    in0=xf[:], in1=bt1_sb[:], op=AluOp.add)
            xsq = xwp.tile([P, D], BF16, tag="xsq")
            nc.vector.tensor_scalar(
                out=xsq[:], in0=x1_sb[:, w * D:(w + 1) * D],
                scalar1=dinvT_sb[:, w:w + 1], scalar2=None, op0=AluOp.mult)
            # transpose xs1 via identity matmul, then h2 = xs1 @ W2
            pstt = pools["psb"].tile([P, 512], F32, name="pstt", tag="psb")
            pst = pstt[:D, :P]
            nc.tensor.matmul(pst, lhsT=xsq[:], rhs=ident[:],
                             start=True, stop=True)
            xst = xwp.tile([D, P], BF16, tag="xst")
            nc.scalar.copy(out=xst[:], in_=pst)
            ph2t = pools["psb"].tile([P, 512], F32, name="ph2t", tag="psb")
            ph2 = ph2t[:, :K]
            nc.tensor.matmul(ph2, lhsT=xst[:], rhs=w2_sb[:D, :],
                             start=True, stop=True)
            nc.vector.tensor_copy(out=g2_shard[:, w * K:(w + 1) * K],
                                  in_=ph2)
            h8 = hwp.tile([P, D], F32)
            nc.scalar.copy(out=h8[:, :K], in_=ph2)
            nc.scalar.activation(out=h8[:, K:], in_=ph2,
                                 func=ActFn.Copy, scale=0.0)
            hw_ = nc.sync.dma_start(out=ag2_in[w * P:(w + 1) * P, :],
                                    in_=h8[:])
            h2_writes.append(hw_)

        def phase2(r, ag1):
            # ---- conv1 edge phase
            state["h2_writes"] = []
            _edge_phase(nc, pools, pr, gtab1, idxrep_d, dstloc_sb, iota_sb,
                        ag1, D, D, finish1)
            ag2 = nc.gpsimd.collective_compute(
                "AllGather", AluOp.bypass,
                replica_groups=[list(range(pr.n_cores))],
                ins=[ag2_in[:, :]], outs=[gtab2[:, :]])
            for w in state["h2_writes"]:
                add_dep_helper(ag2.ins, w.ins, sync=True, reason="ag2 after h2")
            return ag2

        def finish2(w, ps):
            plq = state["plq"]
            t1 = xwp.tile([P, K], F32, tag="t1")
            nc.vector.tensor_tensor(out=t1[:], in0=ps[:],
                                    in1=g2_shard[:, w * K:(w + 1) * K],
                                    op=AluOp.add)
            sl = xwp.tile([P, K], F32, tag="xf")
            nc.vector.tensor_scalar(
                out=sl[:], in0=t1[:], scalar1=dinvT_sb[:, w:w + 1],
                scalar2=None, op0=AluOp.mult)
            sl2 = xwp.tile([P, K], F32, tag="sl2")
            nc.vector.tensor_tensor(out=sl2[:], in0=sl[:], in1=bt2_sb[:],
                                    op=AluOp.add)
            ex = xwp.tile([P, K], F32, tag="ex")
            nc.scalar.activation(out=ex[:], in_=sl2[:], func=ActFn.Exp)
            sm = xwp.tile([P, 1], F32, tag="sm")
            nc.vector.tensor_reduce(out=sm[:], in_=ex[:],
                                    axis=mybir.AxisListType.X, op=AluOp.add)
            rc = xwp.tile([P, 1], F32, tag="rc")
            nc.vector.reciprocal(out=rc[:], in_=sm[:])
            sq = xwp.tile([P, K], BF16, tag="sq")
            nc.vector.tensor_scalar(
                out=sq[:], in0=ex[:], scalar1=rc[:, :1], scalar2=None,
                op0=AluOp.mult)
            ob = xwp.tile([P, P], BF16, tag="ob")
            nc.vector.tensor_tensor(
                out=ob[:], in0=bloc_sb[:, w:w + 1].to_broadcast([P, P]),
                in1=iota_sb[:], op=AluOp.is_equal)
            outer = hwp.tile([P, KD], BF16)
            nc.vector.tensor_tensor(
                out=outer[:].rearrange("p (k j) -> p k j", j=D),
                in0=sq[:].unsqueeze(2).to_broadcast([P, K, D]),
                in1=x1_sb[:, w * D:(w + 1) * D]
                    .unsqueeze(1).to_broadcast([P, K, D]),
                op=AluOp.mult)
            for i in range(NQ):
                nc.tensor.matmul(plq[i][:], lhsT=ob[:],
                                 rhs=outer[:, i * 512:(i + 1) * 512],
                                 start=(w == 0), stop=(w == WPC - 1))

        def phase3(r, ag2):
            # ---- conv2 edge phase + pooling accumulation + ReduceScatter
            state["plq"] = [
                plq_pool.tile([P, 512], F32, name=f"plq{i}", tag=f"plq{i}")
                for i in range(NQ)]
            _edge_phase(nc, pools, pr, gtab2, idxrep_d, dstloc_sb, iota_sb,
                        ag2, D, K, finish2)
            pool_sb = cp.tile([P, KD], F32, name="pool_sb", tag="pool_sb")
            for i in range(NQ):
                nc.vector.tensor_copy(out=pool_sb[:, i * 512:(i + 1) * 512],
                                      in_=state["plq"][i][:])
            rw = nc.sync.dma_start(out=rs_in[:, :], in_=pool_sb[:])
            rs = nc.gpsimd.collective_compute(
                "ReduceScatter", AluOp.add,
                replica_groups=[list(range(pr.n_cores))],
                ins=[rs_in[:, :]], outs=[rs_out[:, :]])
            add_dep_helper(rs.ins, rw.ins, sync=True, reason="rs after pool")
            return rs

        rs = None
        for r in range(reps):
            ag1 = phase1(r)
            if rs is not None:  # serialize reps (slope microbenchmark)
                add_dep_helper(ag1.ins, rs.ins, sync=True, reason="rep chain")
            ag2 = phase2(r, ag1)
            rs = phase3(r, ag2)

        out_sb = cp.tile([P // 8, KD], F32, name="out_sb", tag="out_sb")
        rd = nc.sync.dma_start(out=out_sb[:], in_=rs_out[:, :])
        add_dep_helper(rd.ins, rs.ins, sync=True, reason="read after rs")
        nc.sync.dma_start(out=pool_d[:], in_=out_sb[:])
    nc.compile()
    if split:
        _split_waits(nc)
    return nc


# =========================================================================
# runner + glue
# =========================================================================

_EXEC_CACHE = {}


def exec_spmd(nc, in_maps):
    """Execute a prebuilt Bass module on len(in_maps) cores via PJRT.

    Mirrors concourse.bass2jax.run_bass_via_pjrt, but (a) caches the jitted
    callable per-module so repeated runs don't re-trace/re-compile XLA, and
    (b) fetches each output as ONE global [n_cores*rows, cols] array (one
    device round-trip) instead of per-core sliced fetches.
    Returns {name: global np.ndarray} with per-core rows concatenated.
    """
    import jax
    from jax.sharding import Mesh, PartitionSpec
    from jax.experimental.shard_map import shard_map
    from concourse import bass2jax, mybir as _mybir
    from concourse.bass2jax import (_bass_exec_p, install_neuronx_cc_hook,
                                    partition_id_tensor)

    n_cores = len(in_maps)
    key = id(nc)
    if key not in _EXEC_CACHE:
        install_neuronx_cc_hook()
        assert nc.dbg_addr is None or not nc.dbg_callbacks
        partition_name = (nc.partition_id_tensor.name
                          if nc.partition_id_tensor else None)
        in_names, out_names, out_avals, zero_outs = [], [], [], []
        for alloc in nc.m.functions[0].allocations:
            if not isinstance(alloc, _mybir.MemoryLocationSet):
                continue
            name = alloc.memorylocations[0].name
            if alloc.kind == "ExternalInput":
                if name != partition_name:
                    in_names.append(name)
            elif alloc.kind == "ExternalOutput":
                shape = tuple(alloc.tensor_shape)
                dtype = _mybir.dt.np(alloc.dtype)
                out_names.append(name)
                out_avals.append(jax.core.ShapedArray(shape, dtype))
                zero_outs.append(np.zeros(shape, dtype))
        n_params = len(in_names)
        all_in = list(in_names) + list(out_names)
        if partition_name is not None:
            all_in.append(partition_name)
        donate = tuple(range(n_params, n_params + len(out_avals)))

        def _body(*args):
            operands = list(args)
            if partition_name is not None:
                operands.append(partition_id_tensor())
            return tuple(_bass_exec_p.bind(
                *operands, out_avals=tuple(out_avals), in_names=tuple(all_in),
                out_names=tuple(out_names), lowering_input_output_aliases=(),
                sim_require_finite=True, sim_require_nnan=True, nc=nc))

        mesh = Mesh(np.asarray(jax.devices()[:n_cores]), ("core",))
        specs = (PartitionSpec("core"),) * (n_params + len(out_avals))
        fn = jax.jit(
            shard_map(_body, mesh=mesh, in_specs=specs,
                      out_specs=(PartitionSpec("core"),) * len(out_names),
                      check_rep=False),
            donate_argnums=donate, keep_unused=True)
        _EXEC_CACHE[key] = (fn, in_names, out_names, zero_outs)

    fn, in_names, out_names, zero_outs = _EXEC_CACHE[key]
    concat_in = [np.concatenate([np.asarray(m[nm]) for m in in_maps], axis=0)
                 for nm in in_names]
    concat_zeros = [np.zeros((n_cores * z.shape[0], *z.shape[1:]), z.dtype)
                    for z in zero_outs]
    out_arrs = fn(*concat_in, *concat_zeros)
    return {nm: np.asarray(a) for nm, a in zip(out_names, out_arrs)}


def kernel(x_in, edge_index, batch, W1, b1, W2, b2):
    n_cores = 8
    pr = preprocess(x_in, edge_index, batch, W1, b1, W2, b2, n_cores)
    nc = build(pr)
    out = exec_spmd(nc, make_in_maps(pr))
    return np.ascontiguousarray(
        out["pool"].reshape(pr.B, pr.K, pr.D).astype(np.float32))
